# revision 1
# baseline (speedup 1.0000x reference)
"""Trainium2 Bass kernel for nn_Message_gcn (2-layer RGCN + attention HypergraphConv + info-exchange MLP).

Sharding: pure data parallelism — batch 32 split as 4 samples on each of 8 NeuronCores,
per-layer weights replicated on every core.

Per-core algorithm (mathematically identical to the reference, restructured for the PE):
  - attention logits a_n / a_e computed via host-folded vectors u_x/u_e ([C,HH] = w_lin
    reshaped * att summed over the output channel), so the [M,C]@[C,4C] "el" matmul is
    never materialized.
  - softmax over incident hyperedges runs un-masked (max over all entries) and the mask
    is applied multiplicatively after exp() — identical by shift invariance since the
    global hyperedge keeps every row non-empty.
  - 1/deg, 1/|e|, 0.25/D(v) are folded into the adjacency columns / alpha operands so
    aggregations are plain matmuls with fp32 PSUM accumulation.
  - matmul operands use float32r (full-rate fp32 on the PE); the tiny info-exchange MLP
    (2 x [1024x1024] weights per layer, batched over the 4 samples) runs in bf16.
"""

import sys

sys.path.insert(0, "/opt/trn_rl_repo")

from contextlib import ExitStack

import numpy as np
import ml_dtypes

import concourse.bass as bass
import concourse.tile as tile
from concourse import bacc, mybir
from concourse.bass_utils import run_bass_kernel_spmd

BS, N, E, C, HH, L = 32, 256, 64, 512, 4, 2
M = E + 1
NCORES = 8
BSL = BS // NCORES          # samples per core
NB = N // 128               # node partition tiles
CT = C // 128               # channel partition tiles
C2 = 2 * C
KT2 = C2 // 128             # 2C partition tiles (ie)

f32 = mybir.dt.float32
f32r = mybir.dt.float32r
bf16 = mybir.dt.bfloat16
i32 = mybir.dt.int32
AF = mybir.ActivationFunctionType
ALU = mybir.AluOpType
AX = mybir.AxisListType


def _ins0(sl: bass.AP, count: int, pos: int) -> bass.AP:
    """Insert a 0-stride (broadcast) dim of `count` into an AP's free dims at
    position `pos` (0 = right after the partition dim, -1 = innermost)."""
    ap = [list(p) for p in sl.ap]
    if pos == -1:
        pos = len(ap) - 1
    ap.insert(1 + pos, [0, count])
    return bass.AP(tensor=sl.tensor, offset=sl.offset, ap=ap)


def _rep(sl: bass.AP, count: int) -> bass.AP:
    """[P, 1] AP -> [P, count] via 0-stride repeat of the free dim."""
    ap = [list(p) for p in sl.ap]
    assert ap[-1][1] == 1
    ap[-1] = [0, count]
    return bass.AP(tensor=sl.tensor, offset=sl.offset, ap=ap)


def _zeros_ap(sl: bass.AP, shape) -> bass.AP:
    """All-0-stride AP (scalar broadcast to `shape`, partition dim first)."""
    return bass.AP(tensor=sl.tensor, offset=sl.offset, ap=[[0, n] for n in shape])


def build_module():
    nc = bacc.Bacc("TRN2", target_bir_lowering=False, debug=False)

    # ---- DRAM I/O ----
    d_x0T = nc.dram_tensor("x0T", [BSL, C, N], bf16, kind="ExternalInput")
    d_eaT = nc.dram_tensor("eaT", [BSL, C, M], bf16, kind="ExternalInput")
    d_s2w = nc.dram_tensor("s2w", [BSL, N, E], i32, kind="ExternalInput")
    d_aug = nc.dram_tensor("aug", [BSL, N, N], i32, kind="ExternalInput")
    d_pun = nc.dram_tensor("pun", [BSL, N, N], i32, kind="ExternalInput")
    d_wlin = nc.dram_tensor("wlin", [L, C, HH * C], bf16, kind="ExternalInput")
    d_ux = nc.dram_tensor("ux", [L, C, HH], bf16, kind="ExternalInput")
    d_ue = nc.dram_tensor("ue", [L, C, HH], bf16, kind="ExternalInput")
    d_wcat = nc.dram_tensor("wcat", [L, C, 3 * C], bf16, kind="ExternalInput")
    d_iw1 = nc.dram_tensor("iw1", [L, C2, C2], bf16, kind="ExternalInput")
    d_iw2 = nc.dram_tensor("iw2", [L, C2, C2], bf16, kind="ExternalInput")
    d_brg = nc.dram_tensor("brg", [L, C], bf16, kind="ExternalInput")
    d_bhg = nc.dram_tensor("bhg", [L, C], bf16, kind="ExternalInput")
    d_ib1 = nc.dram_tensor("ib1", [L, C2], bf16, kind="ExternalInput")
    d_ib2 = nc.dram_tensor("ib2", [L, C2], bf16, kind="ExternalInput")
    d_ones = nc.dram_tensor("onesc", [1, 128], bf16, kind="ExternalInput")
    d_eyer = nc.dram_tensor("eyer", [128, 128], f32, kind="ExternalInput")
    d_eyeb = nc.dram_tensor("eyeb", [128, 128], bf16, kind="ExternalInput")
    d_onesb = nc.dram_tensor("onesb", [1, 4], bf16, kind="ExternalInput")
    d_outr = nc.dram_tensor("outr", [BSL, N, C], f32, kind="ExternalOutput")
    d_outh = nc.dram_tensor("outh", [BSL, N, C], f32, kind="ExternalOutput")

    with ExitStack() as ctx:
        tc = ctx.enter_context(tile.TileContext(nc))
        const = ctx.enter_context(tc.tile_pool(name="const", bufs=1))
        wts = ctx.enter_context(tc.tile_pool(name="wts", bufs=1))
        xst = ctx.enter_context(tc.tile_pool(name="xst", bufs=8))
        graph = ctx.enter_context(tc.tile_pool(name="graph", bufs=BSL))
        p1 = ctx.enter_context(tc.tile_pool(name="p1", bufs=1))
        p2 = ctx.enter_context(tc.tile_pool(name="p2", bufs=2))
        p3 = ctx.enter_context(tc.tile_pool(name="p3", bufs=3))
        ps = ctx.enter_context(tc.tile_pool(name="ps", bufs=6, space="PSUM"))
        psA = ctx.enter_context(tc.tile_pool(name="psA", bufs=2, space="PSUM"))

        # x-state input loads first (everything at layer 0 hangs off them)
        Hbs = [None] * BSL
        invDqs = [None] * BSL
        invBs = [None] * BSL
        xrTs = [None] * BSL
        xhTs = [None] * BSL
        for s0_ in range(BSL):
            x0T_t = xst.tile([128, CT, N], bf16, tag="xst")
            nc.sync.dma_start(x0T_t[:], d_x0T[s0_].rearrange("(ct p) n -> p ct n", p=128))
            xrTs[s0_] = x0T_t
            xhTs[s0_] = x0T_t

        # ---- constants ----
        ones_row = const.tile([1, 128], bf16)
        nc.sync.dma_start(ones_row[:], d_ones[:])
        ones_col = const.tile([128, 1], bf16)
        nc.sync.dma_start(ones_col[:], d_ones[0:1, :].rearrange("o p -> p o"))
        identr = const.tile([128, 128], f32)
        nc.sync.dma_start(identr[:], d_eyer[:])
        identb = const.tile([128, 128], bf16)
        nc.sync.dma_start(identb[:], d_eyeb[:])
        ones4b = const.tile([1, 4], bf16)
        nc.sync.dma_start(ones4b[:], d_onesb[:])



        for l in range(L):
            # tiny attention weights first: they unblock sample-0's logits
            ux_t = p2.tile([128, CT, HH], bf16, tag="ux")
            nc.sync.dma_start(ux_t[:], d_ux[l].rearrange("(ct p) h -> p ct h", p=128))
            ue_t = p2.tile([128, CT, HH], bf16, tag="ue")
            nc.sync.dma_start(ue_t[:], d_ue[l].rearrange("(ct p) h -> p ct h", p=128))

            def stageA(s):
                """Adjacency prep (layer 0 only) + attention logits for sample s
                (PE-dense, emitted one sample ahead so the alpha chain of the
                previous sample overlaps)."""
                if l == 0:
                    Hinc_t = p1.tile([128, NB, M], bf16, tag="Hinc")
                    nc.sync.dma_start(Hinc_t[:, :, 0:1], _zeros_ap(d_ones[0:1, 0:1], [128, NB, 1]))
                    nc.gpsimd.dma_start(Hinc_t[:, :, 1:M], d_s2w[s].rearrange("(t p) e -> p t e", p=128))
                    # additive mask: 0 where incident, -50 where not
                    Hb_t = graph.tile([128, NB, M], f32)
                    nc.vector.tensor_scalar(Hb_t[:], Hinc_t[:], 50.0, 50.0, op0=ALU.mult, op1=ALU.subtract)
                    Hbs[s] = Hb_t

                    Dn = p1.tile([128, NB], f32, tag="Dn")
                    nc.vector.tensor_reduce(Dn[:], Hinc_t[:], axis=AX.X, op=ALU.add)
                    eqD = p1.tile([128, NB], f32, tag="eqD")
                    nc.vector.tensor_scalar(eqD[:], Dn[:], 0.0, None, op0=ALU.is_equal)
                    invDq_t = graph.tile([128, NB], f32)
                    nc.vector.tensor_add(invDq_t[:], Dn[:], eqD[:])
                    nc.vector.reciprocal(invDq_t[:], invDq_t[:])
                    nc.vector.tensor_sub(invDq_t[:], invDq_t[:], eqD[:])
                    nc.vector.tensor_scalar(invDq_t[:], invDq_t[:], 0.25, None, op0=ALU.mult)
                    invDqs[s] = invDq_t

                    Be_ps = psA.tile([M, 1], f32, tag="psA")
                    for it in range(NB):
                        nc.tensor.matmul(Be_ps[:], Hinc_t[:, it, :], ones_col[:],
                                         start=(it == 0), stop=(it == NB - 1))
                    Be = p1.tile([M, 1], f32, tag="Be")
                    nc.vector.tensor_copy(Be[:], Be_ps[:])
                    eqB = p1.tile([M, 1], f32, tag="eqB")
                    nc.vector.tensor_scalar(eqB[:], Be[:], 0.0, None, op0=ALU.is_equal)
                    invB_t = graph.tile([M, 1], f32)
                    nc.vector.tensor_add(invB_t[:], Be[:], eqB[:])
                    nc.vector.reciprocal(invB_t[:], invB_t[:])
                    nc.vector.tensor_sub(invB_t[:], invB_t[:], eqB[:])
                    invBs[s] = invB_t

                xhT = xhTs[s]
                if l == 0:
                    # typed adjacency (cast DMAs) and its in-degree columns
                    Af = graph.tile([128, 2, NB, N], bf16)
                    nc.gpsimd.dma_start(Af[:, 1, :, :], d_aug[s].rearrange("(t p) j -> p t j", p=128))
                    nc.gpsimd.dma_start(Af[:, 0, :, :], d_pun[s].rearrange("(t p) j -> p t j", p=128))
                    onem = p1.tile([128, NB, N], f32, tag="gtmp")
                    nc.vector.tensor_scalar(onem[:], Af[:, 1, :, :], -1.0, 1.0, op0=ALU.mult, op1=ALU.add)
                    nc.vector.tensor_mul(Af[:, 0, :, :], Af[:, 0, :, :], onem[:])
                    Afs[s] = Af

                # attention logits a_n (columns) / a_e (row gather)
                an_ps = psA.tile([128, NB * HH], f32, tag="psA")
                for nb in range(NB):
                    for ct in range(CT):
                        nc.tensor.matmul(an_ps[:, nb * HH : (nb + 1) * HH],
                                         xhT[:, ct, nb * 128 : (nb + 1) * 128],
                                         ux_t[:, ct, :],
                                         start=(ct == 0), stop=(ct == CT - 1))
                an_sb = p1.tile([128, NB * HH], f32, tag="ansb")
                nc.scalar.copy(an_sb[:], an_ps[:])

                eaT_t = p1.tile([128, CT, M + 1], bf16, tag="eaT")
                nc.sync.dma_start(eaT_t[:, :, 0:M], d_eaT[s].rearrange("(ct p) m -> p ct m", p=128))
                nc.vector.tensor_copy(eaT_t[:, :, M : M + 1], eaT_t[:, :, M - 1 : M])
                ae_ps = psA.tile([HH, M + 1], f32, tag="psA")
                for ct in range(CT):
                    nc.tensor.matmul(ae_ps[:], ue_t[:, ct, :], eaT_t[:, ct, :],
                                     start=(ct == 0), stop=(ct == CT - 1))
                ae4_sb = p1.tile([HH, M], bf16, tag="ae4sb")
                nc.vector.tensor_copy(ae4_sb[:], ae_ps[:, 0:M])
                ae_row = p1.tile([1, HH, M], bf16, tag="aerow")
                for h, eng in ((0, nc.sync), (1, nc.scalar), (2, nc.gpsimd), (3, nc.sync)):
                    eng.dma_start(ae_row[:, h, :], ae4_sb[h : h + 1, :])
                ab_ps = psA.tile([128, HH, M], f32, tag="psA")
                nc.tensor.matmul(ab_ps[:], ones_row[:], ae_row[0:1, :, :], start=True, stop=True)
                ab_sb = p1.tile([128, HH, M], f32, tag="absb")
                nc.scalar.copy(ab_sb[:], ab_ps[:])

                if l == 0:
                    # in-degree rows -> transpose into per-target columns -> guarded 1/deg
                    Af = Afs[s]
                    deg_ps = psA.tile([1, 2, N], f32, tag="psA")
                    for r in range(2):
                        for it in range(NB):
                            nc.tensor.matmul(deg_ps[:, r, :], ones_col[:], Af[:, r, it, :],
                                             start=(it == 0), stop=(it == NB - 1))
                    degrow = p1.tile([1, 2, N], bf16, tag="degrow")
                    nc.scalar.copy(degrow[:], deg_ps[:])
                    degc_ps = psA.tile([128, 2 * NB, 2], bf16, tag="psA")
                    for r in range(2):
                        for jb in range(NB):
                            nc.tensor.transpose(degc_ps[:, r * NB + jb, 0:1],
                                                degrow[0:1, r, jb * 128 : (jb + 1) * 128],
                                                identb[0:1, 0:1])
                    eqc = p1.tile([128, 2 * NB], f32, tag="eqc")
                    nc.vector.tensor_scalar(eqc[:], degc_ps[:, :, 0], 0.0, None, op0=ALU.is_equal)
                    ivc = graph.tile([128, 2 * NB], f32)
                    nc.vector.tensor_add(ivc[:], degc_ps[:, :, 0], eqc[:])
                    nc.vector.reciprocal(ivc[:], ivc[:])
                    nc.vector.tensor_sub(ivc[:], ivc[:], eqc[:])
                    ivcs[s] = ivc
                return dict(an_sb=an_sb, ab_sb=ab_sb)

            ctxT = p1.tile([128, 2 * CT, BSL], bf16, tag="ctxT")
            if l == 0:
                Afs = [None] * BSL
                ivcs = [None] * BSL
            stA = [None] * BSL
            stA[0] = stageA(0)

            # ---- bulk layer weights (after sample-0 prep so its inputs go first) ----
            wlin_t = p2.tile([128, CT, HH * C], bf16, tag="wlin")
            dw = d_wlin[l].rearrange("(ct p) k -> p ct k", p=128)
            for h in range(HH):
                nc.scalar.dma_start(wlin_t[:, :, h * C : (h + 1) * C], dw[:, :, h * C : (h + 1) * C])
            wcat_t = p2.tile([128, CT, 3 * C], bf16, tag="wcat")
            dc = d_wcat[l].rearrange("(ct p) k -> p ct k", p=128)
            for r3 in range(3):
                nc.scalar.dma_start(wcat_t[:, :, r3 * C : (r3 + 1) * C], dc[:, :, r3 * C : (r3 + 1) * C])
            brg_row = p2.tile([1, C], bf16, tag="brg")
            nc.sync.dma_start(brg_row[:], d_brg[l : l + 1, :])
            bhg_row = p2.tile([1, C], bf16, tag="bhg")
            nc.sync.dma_start(bhg_row[:], d_bhg[l : l + 1, :])
            ib1_row = p2.tile([1, C2], bf16, tag="ib1")
            nc.sync.dma_start(ib1_row[:], d_ib1[l : l + 1, :])
            ib2_row = p2.tile([1, C2], bf16, tag="ib2")
            nc.sync.dma_start(ib2_row[:], d_ib2[l : l + 1, :])
            for s in range(BSL):
                if s + 1 < BSL:
                    stA[s + 1] = stageA(s + 1)
                xhT = xhTs[s]
                xrT = xrTs[s]
                Af = Afs[s]
                ivc = ivcs[s]
                an_sb = stA[s]["an_sb"]
                ab_sb = stA[s]["ab_sb"]

                # ---- alpha: leaky + additive-masked softmax over both node blocks ----
                t1 = p2.tile([128, NB, HH, M], f32, tag="t1")
                an_v = _ins0(an_sb[:].rearrange("p (n h) -> p n h", n=NB), M, -1)
                nc.vector.tensor_tensor(t1[:], _ins0(ab_sb[:], NB, 0), an_v, op=ALU.add)
                nc.vector.scalar_tensor_tensor(t1[:], t1[:], 0.2, t1[:], op0=ALU.mult, op1=ALU.max)
                nc.vector.tensor_tensor(t1[:], t1[:], _ins0(Hbs[s][:], HH, 1), op=ALU.add)
                nmax = p2.tile([128, NB, HH], f32, tag="nmax")
                nc.vector.tensor_reduce(nmax[:], t1[:], axis=AX.X, op=ALU.max, negate=True)
                ssum = p2.tile([128, NB, HH], f32, tag="ssum")
                for nb in range(NB):
                    for h in range(HH):
                        nc.scalar.activation(t1[:, nb, h, :], t1[:, nb, h, :], AF.Exp,
                                             bias=nmax[:, nb, h : h + 1],
                                             accum_out=ssum[:, nb, h : h + 1])
                rs = p2.tile([128, NB, HH], f32, tag="rs")
                nc.vector.reciprocal(rs[:], ssum[:])
                rcol2 = p2.tile([128, NB, HH], f32, tag="rcol2")
                nc.vector.tensor_tensor(rcol2[:], rs[:], _ins0(invDqs[s][:], HH, -1), op=ALU.mult)
                alpha = p2.tile([128, NB, HH, M], bf16, tag="alpha")
                nc.vector.tensor_tensor(alpha[:], t1[:], _ins0(rs[:], M, -1), op=ALU.mult)
                a2b = p2.tile([128, NB, HH, M], bf16, tag="a2b")
                nc.vector.tensor_tensor(a2b[:], t1[:], _ins0(rcol2[:], M, -1), op=ALU.mult)

                # ---- hypergraph conv ----
                alpha3T = p2.tile([M, HH, N], bf16, tag="alpha3T")
                msg = p2.tile([M, HH, C], bf16, tag="msg")
                xlhs = {}
                for h in range(2):
                    xlh = p3.tile([128, NB, C], bf16, tag="xlh")
                    for nb in range(NB):
                        xl_ps = ps.tile([128, C], f32, tag="ps")
                        for ct in range(CT):
                            nc.tensor.matmul(xl_ps[:],
                                             xhT[:, ct, nb * 128 : (nb + 1) * 128],
                                             wlin_t[:, ct, h * C : (h + 1) * C],
                                             start=(ct == 0), stop=(ct == CT - 1))
                        if (h + nb) % 2 == 0:
                            nc.vector.tensor_copy(xlh[:, nb, :], xl_ps[:])
                        else:
                            nc.scalar.copy(xlh[:, nb, :], xl_ps[:])
                    xlhs[h] = xlh
                # alpha2 transposes (DVE chain above has had the xl block to complete)
                for nb in range(NB):
                    for h in range(HH):
                        aT_ps = ps.tile([M, 128], bf16, tag="ps")
                        nc.tensor.transpose(aT_ps[:], a2b[:, nb, h, :], identb[:])
                        nc.scalar.activation(alpha3T[:, h, nb * 128 : (nb + 1) * 128],
                                             aT_ps[:], AF.Copy, scale=invBs[s][:, 0:1])
                for h in range(HH):
                    if h >= 2:
                        xlh = p3.tile([128, NB, C], bf16, tag="xlh")
                        for nb in range(NB):
                            xl_ps = ps.tile([128, C], f32, tag="ps")
                            for ct in range(CT):
                                nc.tensor.matmul(xl_ps[:],
                                                 xhT[:, ct, nb * 128 : (nb + 1) * 128],
                                                 wlin_t[:, ct, h * C : (h + 1) * C],
                                                 start=(ct == 0), stop=(ct == CT - 1))
                            if (h + nb) % 2 == 0:
                                nc.vector.tensor_copy(xlh[:, nb, :], xl_ps[:])
                            else:
                                nc.scalar.copy(xlh[:, nb, :], xl_ps[:])
                        xlhs[h] = xlh
                    xlh = xlhs[h]
                    msg_ps = ps.tile([M, C], f32, tag="ps")
                    for nb in range(NB):
                        nc.tensor.matmul(msg_ps[:], alpha[:, nb, h, :], xlh[:, nb, :],
                                         start=(nb == 0), stop=(nb == NB - 1))
                    if h % 2 == 0:
                        nc.scalar.copy(msg[:, h, :], msg_ps[:])
                    else:
                        nc.vector.tensor_copy(msg[:, h, :], msg_ps[:])

                outh_t = p1.tile([128, NB, C], f32, tag="outh_t")
                for nb in range(NB):
                    oh_ps = ps.tile([128, C], f32, tag="ps")
                    for h in range(HH):
                        nc.tensor.matmul(oh_ps[:], alpha3T[:, h, nb * 128 : (nb + 1) * 128],
                                         msg[:, h, :], start=(h == 0), stop=False)
                    nc.tensor.matmul(oh_ps[:], ones_row[:], bhg_row[:], start=False, stop=True)
                    nc.scalar.activation(outh_t[:, nb, :], oh_ps[:], AF.Relu)

                # ---- RGCN ----
                xw = p2.tile([128, NB, 2, C], bf16, tag="xw")
                for r in range(2):
                    for nb in range(NB):
                        xw_ps = ps.tile([128, C], f32, tag="ps")
                        for ct in range(CT):
                            nc.tensor.matmul(xw_ps[:],
                                             xrT[:, ct, nb * 128 : (nb + 1) * 128],
                                             wcat_t[:, ct, r * C : (r + 1) * C],
                                             start=(ct == 0), stop=(ct == CT - 1))
                        if (r + nb) % 2 == 0:
                            nc.vector.tensor_copy(xw[:, nb, r, :], xw_ps[:])
                        else:
                            nc.scalar.copy(xw[:, nb, r, :], xw_ps[:])

                outr_t = p1.tile([128, NB, C], f32, tag="outr_t")
                for jb in range(NB):
                    a0_ps = ps.tile([128, C], f32, tag="ps")
                    for it in range(NB):
                        nc.tensor.matmul(a0_ps[:], Af[:, 0, it, jb * 128 : (jb + 1) * 128],
                                         xw[:, it, 0, :], start=(it == 0), stop=(it == NB - 1))
                    a1_ps = ps.tile([128, C], f32, tag="ps")
                    for it in range(NB):
                        nc.tensor.matmul(a1_ps[:], Af[:, 1, it, jb * 128 : (jb + 1) * 128],
                                         xw[:, it, 1, :], start=(it == 0), stop=(it == NB - 1))
                    rb_ps = ps.tile([128, C], f32, tag="ps")
                    for ct in range(CT):
                        nc.tensor.matmul(rb_ps[:],
                                         xrT[:, ct, jb * 128 : (jb + 1) * 128],
                                         wcat_t[:, ct, 2 * C : 3 * C],
                                         start=(ct == 0), stop=False)
                    nc.tensor.matmul(rb_ps[:], ones_row[:], brg_row[:], start=False, stop=True)
                    tb = p1.tile([128, C], f32, tag="tb")
                    nc.vector.tensor_scalar(tb[:], a0_ps[:], ivc[:, jb : jb + 1], None, op0=ALU.mult)
                    nc.vector.scalar_tensor_tensor(tb[:], a1_ps[:], ivc[:, NB + jb : NB + jb + 1],
                                                   tb[:], op0=ALU.mult, op1=ALU.add)
                    nc.vector.tensor_tensor(tb[:], rb_ps[:], tb[:], op=ALU.add)
                    nc.scalar.activation(outr_t[:, jb, :], tb[:], AF.Relu)

                # ---- ctx rows into bf16 ctxT columns ----
                ctx_psr = ps.tile([128, CT], f32, tag="ps")
                ctx_psh = ps.tile([128, CT], f32, tag="ps")
                for ct in range(CT):
                    nc.tensor.transpose(ctx_psr[:, ct : ct + 1],
                                        outr_t[0:1, 0, ct * 128 : (ct + 1) * 128],
                                        identr[0:1, 0:1])
                    nc.tensor.transpose(ctx_psh[:, ct : ct + 1],
                                        outh_t[0:1, 0, ct * 128 : (ct + 1) * 128],
                                        identr[0:1, 0:1])
                nc.vector.tensor_copy(ctxT[:, 0:CT, s], ctx_psr[:])
                nc.vector.tensor_copy(ctxT[:, CT : 2 * CT, s], ctx_psh[:])

                if s == 2:
                    iw1_t = wts.tile([128, KT2, C2], bf16, tag="iw1")
                    nc.scalar.dma_start(iw1_t[:], d_iw1[l].rearrange("(kt p) k -> p kt k", p=128))
                    iw2_t = wts.tile([128, KT2, C2], bf16, tag="iw2")
                    nc.scalar.dma_start(iw2_t[:], d_iw2[l].rearrange("(kt p) k -> p kt k", p=128))

                if l == 0:
                    xrT1 = xst.tile([128, CT, N], bf16, tag="xst")
                    xhT1 = xst.tile([128, CT, N], bf16, tag="xst")
                    for src, dst in ((outr_t, xrT1), (outh_t, xhT1)):
                        for nb in range(NB):
                            for ct in range(CT):
                                tp_ps = ps.tile([128, 128], f32, tag="ps")
                                nc.tensor.transpose(tp_ps[:],
                                                    src[:, nb, ct * 128 : (ct + 1) * 128],
                                                    identr[:])
                                if (nb + ct) % 2 == 0:
                                    nc.vector.tensor_copy(dst[:, ct, nb * 128 : (nb + 1) * 128], tp_ps[:])
                                else:
                                    nc.scalar.copy(dst[:, ct, nb * 128 : (nb + 1) * 128], tp_ps[:])
                    xrTs[s] = xrT1
                    xhTs[s] = xhT1
                else:
                    for tsrc, dram in ((outr_t, d_outr), (outh_t, d_outh)):
                        nc.sync.dma_start(dram[s, 1:128, :], tsrc[1:128, 0, :])
                        nc.sync.dma_start(dram[s, 128:N, :], tsrc[:, 1, :])

            # ---- info-exchange MLP over the 4 samples' ctx rows (bf16) ----
            y1 = p1.tile([BSL, C2], bf16, tag="y1")
            for ch in range(2):
                ie_ps = ps.tile([BSL, C], f32, tag="ps")
                for kt in range(KT2):
                    nc.tensor.matmul(ie_ps[:], ctxT[:, kt, :], iw1_t[:, kt, ch * C : (ch + 1) * C],
                                     start=(kt == 0), stop=False)
                nc.tensor.matmul(ie_ps[:], ones4b[:], ib1_row[:, ch * C : (ch + 1) * C],
                                 start=False, stop=True)
                nc.scalar.activation(y1[:, ch * C : (ch + 1) * C], ie_ps[:], AF.Relu)
            ctx2T = p1.tile([128, KT2, BSL], bf16, tag="ctx2T")
            for kt in range(KT2):
                t2_ps = ps.tile([128, BSL], bf16, tag="ps")
                nc.tensor.transpose(t2_ps[:], y1[:, kt * 128 : (kt + 1) * 128],
                                    identb[0:BSL, 0:BSL])
                nc.vector.tensor_copy(ctx2T[:, kt, :], t2_ps[:])
            y2 = p1.tile([BSL, C2], f32, tag="y2")
            for ch in range(2):
                ie2_ps = ps.tile([BSL, C], f32, tag="ps")
                for kt in range(KT2):
                    nc.tensor.matmul(ie2_ps[:], ctx2T[:, kt, :], iw2_t[:, kt, ch * C : (ch + 1) * C],
                                     start=(kt == 0), stop=False)
                nc.tensor.matmul(ie2_ps[:], ones4b[:], ib2_row[:, ch * C : (ch + 1) * C],
                                 start=False, stop=True)
                nc.vector.tensor_copy(y2[:, ch * C : (ch + 1) * C], ie2_ps[:])

            if l == 0:
                for kt in range(KT2):
                    y2T_ps = ps.tile([128, BSL], f32, tag="ps")
                    nc.tensor.transpose(y2T_ps[:], y2[:, kt * 128 : (kt + 1) * 128],
                                        identr[0:BSL, 0:BSL])
                    for s in range(BSL):
                        dst = xrTs[s] if kt < CT else xhTs[s]
                        nc.vector.tensor_copy(dst[:, kt % CT, 0:1], y2T_ps[:, s : s + 1])
            else:
                for s in range(BSL):
                    nc.sync.dma_start(d_outr[s, 0:1, :], y2[s : s + 1, 0:C])
                    nc.sync.dma_start(d_outh[s, 0:1, :], y2[s : s + 1, C:C2])

    nc.compile()
    return nc


_NC = None


def _get_nc():
    global _NC
    if _NC is None:
        _NC = build_module()
    return _NC


def make_in_maps(encoded_spans, SVO_emb, pooled_output, sent2word_adj, aug_adj,
                 punct_graph, w_rel, w_root, b_rgcn, w_lin, att_x, att_e, b_hgcn,
                 ie_w1, ie_b1, ie_w2, ie_b2):
    f = np.float32
    bf = ml_dtypes.bfloat16
    # host-folded attention vectors: u[c,h] = sum_k w_lin[c, h*C+k] * att[h,k]
    wl = np.ascontiguousarray(np.asarray(w_lin, f))                # [L, C, HH*C]
    wl4 = wl.reshape(L, C, HH, C)
    ux = np.einsum("lchk,lhk->lch", wl4, np.asarray(att_x, f))     # [L, C, HH]
    ue = np.einsum("lchk,lhk->lch", wl4, np.asarray(att_e, f))
    wr = np.asarray(w_rel, f)
    wcat = np.concatenate([wr[:, 0], wr[:, 1], np.asarray(w_root, f)], axis=2)
    e_attr = np.concatenate([np.asarray(pooled_output, f)[:, None, :],
                             np.asarray(SVO_emb, f)], axis=1)      # [BS, M, C]
    eaT = np.ascontiguousarray(e_attr.transpose(0, 2, 1))          # [BS, C, M]
    x0T = np.ascontiguousarray(np.asarray(encoded_spans, f).transpose(0, 2, 1))

    shared = {
        "wlin": wl.astype(bf),
        "ux": np.ascontiguousarray(ux).astype(bf),
        "ue": np.ascontiguousarray(ue).astype(bf),
        "wcat": np.ascontiguousarray(wcat).astype(bf),
        "iw1": np.asarray(ie_w1, f).astype(bf),
        "iw2": np.asarray(ie_w2, f).astype(bf),
        "brg": np.asarray(b_rgcn, f).astype(bf),
        "bhg": np.asarray(b_hgcn, f).astype(bf),
        "ib1": np.asarray(ie_b1, f).astype(bf),
        "ib2": np.asarray(ie_b2, f).astype(bf),
        "onesc": np.ones((1, 128), f).astype(bf),
        "eyer": np.eye(128, dtype=f),
        "eyeb": np.eye(128, dtype=f).astype(bf),
        "onesb": np.ones((1, 4), f).astype(bf),
    }
    s2w = np.ascontiguousarray(np.asarray(sent2word_adj, np.int32))
    aug = np.ascontiguousarray(np.asarray(aug_adj, np.int32))
    pun = np.ascontiguousarray(np.asarray(punct_graph, np.int32))

    in_maps = []
    for c in range(NCORES):
        sl = slice(c * BSL, (c + 1) * BSL)
        m = dict(shared)
        m["x0T"] = np.ascontiguousarray(x0T[sl]).astype(bf)
        m["eaT"] = np.ascontiguousarray(eaT[sl]).astype(bf)
        m["s2w"] = s2w[sl]
        m["aug"] = aug[sl]
        m["pun"] = pun[sl]
        in_maps.append(m)
    return in_maps


def run(in_maps, trace=False, **kw):
    nc = _get_nc()
    return run_bass_kernel_spmd(nc, in_maps, list(range(NCORES)), trace=trace, **kw)


def kernel(**inputs):
    in_maps = make_in_maps(**inputs)
    res = run(in_maps)
    x_r = np.concatenate([res.results[c]["outr"] for c in range(NCORES)], axis=0)
    x_h = np.concatenate([res.results[c]["outh"] for c in range(NCORES)], axis=0)
    return x_r.astype(np.float32), x_h.astype(np.float32)



# revision 10
# speedup vs baseline: 1.1001x; 1.1001x over previous
"""Trainium2 Bass kernel for nn_Message_gcn (2-layer RGCN + attention HypergraphConv + info-exchange MLP).

Sharding: pure data parallelism - batch 32 split as 4 samples on each of 8 NeuronCores,
per-layer weights replicated on every core.

v2 layout (PE-continuity focused):
  - graph preprocessing is host-folded: typed adjacency (punct&!aug / aug) is built,
    degree-normalized and shipped as bf16; the hypergraph incidence ships as an additive
    softmax mask (0 / -50), 0.25/D(v) and 1/|e| ship as ready-made columns; layer-0
    node logits a_n and both layers' hyperedge logits a_e (constant across layers)
    are tiny host einsums.
  - per layer, all alpha-independent matmuls (x @ [w_lin | w_rel | w_root]) are emitted
    as one dense per-sample "bulk" block so the PE stays busy (HAM stays at 8/8) while
    DVE/GpSimd/Act run the masked-softmax chains.
  - layer-0 outputs are produced directly in transposed [C, N] layout (the layout layer 1
    consumes): hypergraph out = msg^T @ alpha2^T and RGCN agg = xw^T @ Anorm, with the
    root term applied as an identity-matmul and the bias folded into the PSUM->SBUF
    relu (per-partition Act bias / DVE tensor_scalar).
  - layer-1 outputs are row-major bf16 and stream straight to DRAM; node-0 rows come
    from the info-exchange MLP output.
"""

import sys

sys.path.insert(0, "/opt/trn_rl_repo")

from contextlib import ExitStack

import numpy as np
import ml_dtypes

import concourse.bass as bass
import concourse.tile as tile
from concourse import bacc, mybir
from concourse.bass_utils import run_bass_kernel_spmd

BS, N, E, C, HH, L = 32, 256, 64, 512, 4, 2
M = E + 1
NCORES = 8
BSL = BS // NCORES          # samples per core
NB = N // 128               # node partition tiles
CT = C // 128               # channel partition tiles
C2 = 2 * C
KT2 = C2 // 128             # 2C partition tiles (ie)

f32 = mybir.dt.float32
bf16 = mybir.dt.bfloat16
AF = mybir.ActivationFunctionType
ALU = mybir.AluOpType
AX = mybir.AxisListType


def _ins0(sl: bass.AP, count: int, pos: int) -> bass.AP:
    """Insert a 0-stride (broadcast) dim of `count` into an AP's free dims at
    position `pos` (0 = right after the partition dim, -1 = innermost)."""
    ap = [list(p) for p in sl.ap]
    if pos == -1:
        pos = len(ap) - 1
    ap.insert(1 + pos, [0, count])
    return bass.AP(tensor=sl.tensor, offset=sl.offset, ap=ap)


def _bcast_p(sl: bass.AP, count: int) -> bass.AP:
    """Prepend a 0-stride partition dim of `count` to a DRAM AP (DMA-replicates
    the same source bytes into every partition)."""
    ap = [list(p) for p in sl.ap]
    return bass.AP(tensor=sl.tensor, offset=sl.offset, ap=[[0, count]] + ap)


def build_module():
    nc = bacc.Bacc("TRN2", target_bir_lowering=False, debug=False)

    # ---- DRAM I/O ----
    d_x0T = nc.dram_tensor("x0T", [BSL, C, N], bf16, kind="ExternalInput")
    d_adjn = nc.dram_tensor("adjn", [BSL, 2, N, N], bf16, kind="ExternalInput")
    d_hb = nc.dram_tensor("hbm", [BSL, N, M], bf16, kind="ExternalInput")
    d_ivdq = nc.dram_tensor("ivdq", [BSL, N], f32, kind="ExternalInput")
    d_ivb = nc.dram_tensor("ivb", [BSL, M, 1], f32, kind="ExternalInput")
    d_an0 = nc.dram_tensor("an0", [BSL, N, HH], f32, kind="ExternalInput")
    d_ae = nc.dram_tensor("ae", [L, BSL, HH, M], bf16, kind="ExternalInput")
    d_wlin = nc.dram_tensor("wlin", [L, C, HH * C], bf16, kind="ExternalInput")
    d_wcat = nc.dram_tensor("wcat", [L, C, 3 * C], bf16, kind="ExternalInput")
    d_ux1 = nc.dram_tensor("ux1", [C, HH], bf16, kind="ExternalInput")
    d_iw1 = nc.dram_tensor("iw1", [L, C2, C2], bf16, kind="ExternalInput")
    d_iw2 = nc.dram_tensor("iw2", [L, C2, C2], bf16, kind="ExternalInput")
    d_bcolr = nc.dram_tensor("bcolr", [L, C], f32, kind="ExternalInput")
    d_bcolh = nc.dram_tensor("bcolh", [L, C], f32, kind="ExternalInput")
    d_browr = nc.dram_tensor("browr", [L, C], bf16, kind="ExternalInput")
    d_browh = nc.dram_tensor("browh", [L, C], bf16, kind="ExternalInput")
    d_ib1 = nc.dram_tensor("ib1", [L, C2], bf16, kind="ExternalInput")
    d_ib2 = nc.dram_tensor("ib2", [L, C2], bf16, kind="ExternalInput")
    d_eyeb = nc.dram_tensor("eyeb", [128, 128], bf16, kind="ExternalInput")
    d_ones = nc.dram_tensor("onesr", [1, 128], bf16, kind="ExternalInput")
    d_ones4 = nc.dram_tensor("ones4", [1, 4], bf16, kind="ExternalInput")
    d_outr = nc.dram_tensor("outr", [BSL, N, C], bf16, kind="ExternalOutput")
    d_outh = nc.dram_tensor("outh", [BSL, N, C], bf16, kind="ExternalOutput")

    with ExitStack() as ctx:
        tc = ctx.enter_context(tile.TileContext(nc))
        const = ctx.enter_context(tc.tile_pool(name="const", bufs=1))
        wts = ctx.enter_context(tc.tile_pool(name="wts", bufs=2))
        wie = ctx.enter_context(tc.tile_pool(name="wie", bufs=1))
        xst = ctx.enter_context(tc.tile_pool(name="xst", bufs=8))
        acts = ctx.enter_context(tc.tile_pool(name="acts", bufs=3))
        alph = ctx.enter_context(tc.tile_pool(name="alph", bufs=3))
        msgs = ctx.enter_context(tc.tile_pool(name="msgs", bufs=2))
        outs = ctx.enter_context(tc.tile_pool(name="outs", bufs=2))
        iep = ctx.enter_context(tc.tile_pool(name="iep", bufs=1))
        blk = ctx.enter_context(tc.tile_pool(name="blk", bufs=4, space="PSUM"))
        pst = ctx.enter_context(tc.tile_pool(name="pst", bufs=2, space="PSUM"))
        pmx = ctx.enter_context(tc.tile_pool(name="pmx", bufs=2, space="PSUM"))

        # round-robin over the two PSUM-capable copy engines
        _eng = [0]

        def cpeng():
            _eng[0] ^= 1
            return nc.vector if _eng[0] else nc.scalar

        def relu_bias(dst, src, bias_ap):
            e = cpeng()
            if e is nc.scalar:
                e.activation(dst, src, AF.Relu, bias=bias_ap)
            else:
                e.tensor_scalar(dst, src, bias_ap, 0.0, op0=ALU.add, op1=ALU.max)

        def copy(dst, src, scale=None):
            e = cpeng()
            if e is nc.scalar:
                if scale is None:
                    e.copy(dst, src)
                else:
                    e.activation(dst, src, AF.Copy, scale=scale)
            else:
                if scale is None:
                    e.tensor_copy(dst, src)
                else:
                    e.tensor_scalar(dst, src, scale, None, op0=ALU.mult)

        # ---- input / constant DMAs ----
        x0Ts = [None] * BSL
        for s in range(BSL):
            t = xst.tile([128, CT, N], bf16, tag="xst")
            nc.sync.dma_start(t[:], d_x0T[s].rearrange("(ct p) n -> p ct n", p=128))
            x0Ts[s] = t

        wlins = [None] * L
        wcats = [None] * L
        wlins[0] = wts.tile([128, CT, HH * C], bf16, tag="wlin", name="wlin0")
        dw = d_wlin[0].rearrange("(ct p) k -> p ct k", p=128)
        for h in range(HH):
            nc.sync.dma_start(wlins[0][:, :, h * C : (h + 1) * C], dw[:, :, h * C : (h + 1) * C])

        identb = const.tile([128, 128], bf16)
        nc.sync.dma_start(identb[:], d_eyeb[:])
        ones_row = const.tile([1, 128], bf16)
        nc.sync.dma_start(ones_row[:], d_ones[:])
        ones4 = const.tile([1, 4], bf16)
        nc.sync.dma_start(ones4[:], d_ones4[:])
        ux1 = const.tile([128, CT, HH], bf16)
        nc.sync.dma_start(ux1[:], d_ux1.rearrange("(ct p) h -> p ct h", p=128))

        an0s, abs_, hbs, ivdqs, ivbs, Afns = [], [[], []], [], [], [], []
        for s in range(BSL):
            t = const.tile([128, NB, HH], f32, name=f"an0_{s}")
            nc.scalar.dma_start(t[:], d_an0[s].rearrange("(t p) h -> p t h", p=128))
            an0s.append(t)
            for l in range(L):
                a = const.tile([128, HH, M], bf16, name=f"ab{l}_{s}")
                nc.scalar.dma_start(a[:], _bcast_p(d_ae[l, s], 128))
                abs_[l].append(a)
            t = const.tile([128, NB, M], bf16, name=f"hb_{s}")
            nc.scalar.dma_start(t[:], d_hb[s].rearrange("(t p) m -> p t m", p=128))
            hbs.append(t)
            t = const.tile([128, NB], f32, name=f"ivdq_{s}")
            nc.scalar.dma_start(t[:], d_ivdq[s].rearrange("(t p) -> p t", p=128))
            ivdqs.append(t)
            t = const.tile([M, 1], f32, name=f"ivb_{s}")
            nc.scalar.dma_start(t[:], d_ivb[s])
            ivbs.append(t)
            t = const.tile([128, 2, NB, N], bf16, name=f"afn_{s}")
            eng = nc.gpsimd if s % 2 == 0 else nc.sync
            eng.dma_start(t[:], d_adjn[s].rearrange("r (t p) j -> p r t j", p=128))
            Afns.append(t)

        bcolr, bcolh, browr, browh, ib1r, ib2r = [], [], [], [], [], []
        for l in range(L):
            t = const.tile([128, CT], f32, name=f"bcolr{l}")
            nc.scalar.dma_start(t[:], d_bcolr[l].rearrange("(ct p) -> p ct", p=128))
            bcolr.append(t)
            t = const.tile([128, CT], f32, name=f"bcolh{l}")
            nc.scalar.dma_start(t[:], d_bcolh[l].rearrange("(ct p) -> p ct", p=128))
            bcolh.append(t)
            t = const.tile([1, C], bf16, name=f"browr{l}")
            nc.scalar.dma_start(t[:], d_browr[l : l + 1, :])
            browr.append(t)
            t = const.tile([1, C], bf16, name=f"browh{l}")
            nc.scalar.dma_start(t[:], d_browh[l : l + 1, :])
            browh.append(t)
            t = const.tile([1, C2], bf16, name=f"ib1_{l}")
            nc.scalar.dma_start(t[:], d_ib1[l : l + 1, :])
            ib1r.append(t)
            t = const.tile([1, C2], bf16, name=f"ib2_{l}")
            nc.scalar.dma_start(t[:], d_ib2[l : l + 1, :])
            ib2r.append(t)

        # remaining layer-0 weights, then all layer-1 weights (DMA streams on)
        wcats[0] = wts.tile([128, CT, 3 * C], bf16, tag="wcat", name="wcat0", bufs=1)
        dc = d_wcat[0].rearrange("(ct p) k -> p ct k", p=128)
        for r3 in range(3):
            nc.sync.dma_start(wcats[0][:, :, r3 * C : (r3 + 1) * C], dc[:, :, r3 * C : (r3 + 1) * C])
        wlins[1] = wts.tile([128, CT, HH * C], bf16, tag="wlin", name="wlin1")
        dw = d_wlin[1].rearrange("(ct p) k -> p ct k", p=128)
        for h in range(HH):
            nc.gpsimd.dma_start(wlins[1][:, :, h * C : (h + 1) * C], dw[:, :, h * C : (h + 1) * C])
        iw1_t = wie.tile([128, KT2, C2], bf16, tag="iw1")
        nc.sync.dma_start(iw1_t[:], d_iw1[0].rearrange("(kt p) k -> p kt k", p=128))
        iw2_t = wie.tile([128, KT2, C2], bf16, tag="iw2")
        nc.sync.dma_start(iw2_t[:], d_iw2[0].rearrange("(kt p) k -> p kt k", p=128))

        # ---- per-(layer, sample) pieces ----
        xrTs = list(x0Ts)
        xhTs = list(x0Ts)
        alphas = [None] * BSL
        a2bs = [None] * BSL
        xls = [None] * BSL
        xws = [None] * BSL
        xwrs = [None] * BSL
        an1s = [None] * BSL
        ctxT = None

        def alpha_chain(l, s):
            """Masked softmax over incident hyperedges -> alpha (for msg) and
            a2b = alpha * 0.25/D(v) (for the node-side aggregation).
            GpSimd builds the logits, Act does the exp, DVE the reductions."""
            an_src = an0s[s] if l == 0 else an1s[s]
            t1 = alph.tile([128, NB, HH, M], f32, tag="t1")
            nc.gpsimd.tensor_tensor(t1[:], _ins0(abs_[l][s][:], NB, 0),
                                    _ins0(an_src[:], M, -1), op=ALU.add)
            nc.vector.scalar_tensor_tensor(t1[:], t1[:], 0.2, t1[:],
                                           op0=ALU.mult, op1=ALU.max)
            nc.gpsimd.tensor_tensor(t1[:], t1[:], _ins0(hbs[s][:], HH, 1), op=ALU.add)
            nmax = alph.tile([128, NB, HH], f32, tag="nmax")
            nc.vector.tensor_reduce(nmax[:], t1[:], axis=AX.X, op=ALU.max, negate=True)
            nc.gpsimd.tensor_tensor(t1[:], t1[:], _ins0(nmax[:], M, -1), op=ALU.add)
            nc.scalar.activation(t1[:], t1[:], AF.Exp)
            ssum = alph.tile([128, NB, HH], f32, tag="ssum")
            nc.vector.tensor_reduce(ssum[:], t1[:], axis=AX.X, op=ALU.add)
            rs = alph.tile([128, NB, HH], f32, tag="rs")
            nc.vector.reciprocal(rs[:], ssum[:])
            rcol2 = alph.tile([128, NB, HH], f32, tag="rcol2")
            nc.vector.tensor_tensor(rcol2[:], rs[:], _ins0(ivdqs[s][:], HH, -1), op=ALU.mult)
            al = alph.tile([128, NB, HH, M], bf16, tag="alpha")
            nc.vector.tensor_tensor(al[:], t1[:], _ins0(rs[:], M, -1), op=ALU.mult)
            a2 = alph.tile([128, NB, HH, M], bf16, tag="a2b")
            nc.vector.tensor_tensor(a2[:], t1[:], _ins0(rcol2[:], M, -1), op=ALU.mult)
            alphas[s] = al
            a2bs[s] = a2

        def bulk(l, s):
            """x @ [w_lin | w_rel0 | w_rel1 | w_root]: all alpha-independent
            PE work for (l, s), drained chunk-by-chunk into bf16 SBUF."""
            xl = acts.tile([128, HH, NB, C], bf16, tag="xl")
            xw = acts.tile([128, NB, 2, C], bf16, tag="xw")
            xwr = acts.tile([128, NB, C], bf16, tag="xwr")
            for nt in range(NB):
                for k in range(7):
                    if k < HH:
                        stat, wt, wsl = xhTs[s], wlins[l], slice(k * C, (k + 1) * C)
                        dst = xl[:, k, nt, :]
                    else:
                        stat, wt, wsl = xrTs[s], wcats[l], slice((k - 4) * C, (k - 3) * C)
                        dst = xw[:, nt, k - 4, :] if k < 6 else xwr[:, nt, :]
                    ps = blk.tile([128, C], f32, tag="blk")
                    for ct in range(CT):
                        nc.tensor.matmul(ps[:], stat[:, ct, nt * 128 : (nt + 1) * 128],
                                         wt[:, ct, wsl], start=(ct == 0), stop=(ct == CT - 1))
                    copy(dst, ps[:])
            xls[s], xws[s], xwrs[s] = xl, xw, xwr

        def phase2(l, s):
            """alpha-dependent per-sample work: alpha transposes, msg,
            hypergraph out, RGCN aggregation, ctx column."""
            al, a2 = alphas[s], a2bs[s]
            xl, xw, xwr = xls[s], xws[s], xwrs[s]
            Afn = Afns[s]

            a3T = msgs.tile([M, HH, N], bf16, tag="a3T")
            for nb in range(NB):
                for h in range(HH):
                    tp = pmx.tile([M, 128], bf16, tag="mx", padded_shape=[M, 1024])
                    nc.tensor.transpose(tp[:], a2[:, nb, h, :], identb[:])
                    copy(a3T[:, h, nb * 128 : (nb + 1) * 128], tp[:])

            msg = msgs.tile([M, HH, C], bf16, tag="msg")
            for h in range(HH):
                mp = blk.tile([M, C], f32, tag="blk")
                for nb in range(NB):
                    nc.tensor.matmul(mp[:], al[:, nb, h, :], xl[:, h, nb, :],
                                     start=(nb == 0), stop=(nb == NB - 1))
                copy(msg[:, h, :], mp[:], scale=ivbs[s][:, 0:1])

            if l == 0:
                # transposed outputs -> next-layer state tiles [C, N]
                xhT1 = xst.tile([128, CT, N], bf16, tag="xst")
                xrT1 = xst.tile([128, CT, N], bf16, tag="xst")
                for g in range(CT // 2):
                    ph = pst.tile([128, 2, N], f32, tag="st")
                    for j in range(2):
                        ct = 2 * g + j
                        for h in range(HH):
                            nc.tensor.matmul(ph[:, j, :], msg[:, h, ct * 128 : (ct + 1) * 128],
                                             a3T[:, h, :], start=(h == 0), stop=(h == HH - 1))
                    for j in range(2):
                        ct = 2 * g + j
                        relu_bias(xhT1[:, ct, :], ph[:, j, :], bcolh[l][:, ct : ct + 1])
                for g in range(CT // 2):
                    pr = pst.tile([128, 2, N], f32, tag="st")
                    for j in range(2):
                        dt = 2 * g + j
                        first = True
                        for r in range(2):
                            for it in range(NB):
                                nc.tensor.matmul(pr[:, j, :],
                                                 xw[:, it, r, dt * 128 : (dt + 1) * 128],
                                                 Afn[:, r, it, :], start=first, stop=False)
                                first = False
                        for it in range(NB):
                            nc.tensor.matmul(pr[:, j, it * 128 : (it + 1) * 128],
                                             xwr[:, it, dt * 128 : (dt + 1) * 128],
                                             identb[:], start=False, stop=(it == NB - 1))
                    for j in range(2):
                        dt = 2 * g + j
                        relu_bias(xrT1[:, dt, :], pr[:, j, :], bcolr[l][:, dt : dt + 1])
                xhTs[s] = xhT1
                xrTs[s] = xrT1
                # ctx columns (pre-patch node-0 output)
                nc.vector.tensor_copy(ctxT[:, 0:CT, s : s + 1], xrT1[:, :, 0:1])
                nc.vector.tensor_copy(ctxT[:, CT : 2 * CT, s : s + 1], xhT1[:, :, 0:1])
            else:
                # row-major bf16 outputs, streamed to DRAM (rows 1..N-1)
                outh_t = outs.tile([128, NB, C], bf16, tag="outh")
                for nb in range(NB):
                    po = blk.tile([128, C], f32, tag="blk")
                    for h in range(HH):
                        nc.tensor.matmul(po[:], a3T[:, h, nb * 128 : (nb + 1) * 128],
                                         msg[:, h, :], start=(h == 0), stop=False)
                    nc.tensor.matmul(po[:], ones_row[:], browh[l][:], start=False, stop=True)
                    e = cpeng()
                    if e is nc.scalar:
                        e.activation(outh_t[:, nb, :], po[:], AF.Relu)
                    else:
                        e.tensor_scalar(outh_t[:, nb, :], po[:], 0.0, None, op0=ALU.max)
                outr_t = outs.tile([128, NB, C], bf16, tag="outr")
                for jb in range(NB):
                    po = blk.tile([128, C], f32, tag="blk")
                    first = True
                    for r in range(2):
                        for it in range(NB):
                            nc.tensor.matmul(po[:], Afn[:, r, it, jb * 128 : (jb + 1) * 128],
                                             xw[:, it, r, :], start=first, stop=False)
                            first = False
                    nc.tensor.matmul(po[:], identb[:], xwr[:, jb, :], start=False, stop=False)
                    nc.tensor.matmul(po[:], ones_row[:], browr[l][:], start=False, stop=True)
                    e = cpeng()
                    if e is nc.scalar:
                        e.activation(outr_t[:, jb, :], po[:], AF.Relu)
                    else:
                        e.tensor_scalar(outr_t[:, jb, :], po[:], 0.0, None, op0=ALU.max)
                # ctx rows -> columns
                cps = pmx.tile([128, 2, CT, 2], bf16, tag="mx", padded_shape=[128, 2, CT, 64])
                for ct in range(CT):
                    nc.tensor.transpose(cps[:, 0, ct, 0:1],
                                        outr_t[0:1, 0, ct * 128 : (ct + 1) * 128],
                                        identb[0:1, 0:1])
                    nc.tensor.transpose(cps[:, 1, ct, 0:1],
                                        outh_t[0:1, 0, ct * 128 : (ct + 1) * 128],
                                        identb[0:1, 0:1])
                nc.vector.tensor_copy(
                    ctxT[:, :, s : s + 1].rearrange("p (r ct) o -> p r (ct o)", r=2),
                    cps[:, :, :, 0])
                for tsrc, dram in ((outr_t, d_outr), (outh_t, d_outh)):
                    nc.sync.dma_start(dram[s, 1:128, :], tsrc[1:128, 0, :])
                    nc.sync.dma_start(dram[s, 128:N, :], tsrc[:, 1, :])

        def ie(l):
            """info-exchange MLP over the BSL ctx columns; layer 0 patches the
            state tiles' node-0 column, layer 1 DMAs the node-0 output rows."""
            y1 = iep.tile([BSL, C2], bf16, tag="y1")
            for ch in range(2):
                yp = blk.tile([BSL, C], f32, tag="blk")
                for kt in range(KT2):
                    nc.tensor.matmul(yp[:], ctxT[:, kt, :], iw1_t[:, kt, ch * C : (ch + 1) * C],
                                     start=(kt == 0), stop=False)
                nc.tensor.matmul(yp[:], ones4[:], ib1r[l][:, ch * C : (ch + 1) * C],
                                 start=False, stop=True)
                e = cpeng()
                if e is nc.scalar:
                    e.activation(y1[:, ch * C : (ch + 1) * C], yp[:], AF.Relu)
                else:
                    e.tensor_scalar(y1[:, ch * C : (ch + 1) * C], yp[:], 0.0, None, op0=ALU.max)
            c2T = iep.tile([128, KT2, BSL], bf16, tag="c2T")
            for kt in range(KT2):
                tp = pmx.tile([128, BSL], bf16, tag="mx", padded_shape=[128, 1024])
                nc.tensor.transpose(tp[:], y1[:, kt * 128 : (kt + 1) * 128], identb[0:BSL, 0:BSL])
                copy(c2T[:, kt, :], tp[:])
            y2 = iep.tile([BSL, C2], bf16, tag="y2")
            for ch in range(2):
                yp = blk.tile([BSL, C], f32, tag="blk")
                for kt in range(KT2):
                    nc.tensor.matmul(yp[:], c2T[:, kt, :], iw2_t[:, kt, ch * C : (ch + 1) * C],
                                     start=(kt == 0), stop=False)
                nc.tensor.matmul(yp[:], ones4[:], ib2r[l][:, ch * C : (ch + 1) * C],
                                 start=False, stop=True)
                copy(y2[:, ch * C : (ch + 1) * C], yp[:])
            if l == 0:
                for kt in range(KT2):
                    tp = pmx.tile([128, BSL], bf16, tag="mx", padded_shape=[128, 1024])
                    nc.tensor.transpose(tp[:], y2[:, kt * 128 : (kt + 1) * 128],
                                        identb[0:BSL, 0:BSL])
                    for s in range(BSL):
                        dst = xrTs[s] if kt < CT else xhTs[s]
                        e = cpeng()
                        if e is nc.scalar:
                            e.copy(dst[:, kt % CT, 0:1], tp[:, s : s + 1])
                        else:
                            e.tensor_copy(dst[:, kt % CT, 0:1], tp[:, s : s + 1])
            else:
                for s in range(BSL):
                    nc.sync.dma_start(d_outr[s, 0:1, :], y2[s : s + 1, 0:C])
                    nc.sync.dma_start(d_outh[s, 0:1, :], y2[s : s + 1, C:C2])

        # ================= layer 0 =================
        ctxT = iep.tile([128, 2 * CT, BSL], bf16, tag="ctxT")
        for s in range(BSL):
            alpha_chain(0, s)
        bulk(0, 0)
        bulk(0, 1)
        phase2(0, 0)
        bulk(0, 2)
        phase2(0, 1)
        bulk(0, 3)
        wcats[1] = wts.tile([128, CT, 3 * C], bf16, tag="wcat", name="wcat1", bufs=1)
        dc = d_wcat[1].rearrange("(ct p) k -> p ct k", p=128)
        for r3 in range(3):
            nc.gpsimd.dma_start(wcats[1][:, :, r3 * C : (r3 + 1) * C], dc[:, :, r3 * C : (r3 + 1) * C])
        phase2(0, 2)
        phase2(0, 3)
        ie(0)

        # layer-1 IE weights reuse the same SBUF slots (gated on ie(0) readers)
        iw1_t = wie.tile([128, KT2, C2], bf16, tag="iw1")
        nc.gpsimd.dma_start(iw1_t[:], d_iw1[1].rearrange("(kt p) k -> p kt k", p=128))
        iw2_t = wie.tile([128, KT2, C2], bf16, tag="iw2")
        nc.sync.dma_start(iw2_t[:], d_iw2[1].rearrange("(kt p) k -> p kt k", p=128))

        # ================= layer 1 =================
        ctxT = iep.tile([128, 2 * CT, BSL], bf16, tag="ctxT2")
        for s in range(BSL):
            ap = blk.tile([128, NB * HH], f32, tag="blk", padded_shape=[128, 512])
            for nb in range(NB):
                for ct in range(CT):
                    nc.tensor.matmul(ap[:, nb * HH : (nb + 1) * HH],
                                     xhTs[s][:, ct, nb * 128 : (nb + 1) * 128],
                                     ux1[:, ct, :], start=(ct == 0), stop=(ct == CT - 1))
            an1 = alph.tile([128, NB, HH], f32, tag="an1")
            copy(an1[:].rearrange("p t h -> p (t h)"), ap[:])
            an1s[s] = an1
        for s in range(BSL):
            alpha_chain(1, s)
        bulk(1, 0)
        bulk(1, 1)
        phase2(1, 0)
        bulk(1, 2)
        phase2(1, 1)
        bulk(1, 3)
        phase2(1, 2)
        phase2(1, 3)
        ie(1)

    nc.compile()
    return nc


_NC = None


def _get_nc():
    global _NC
    if _NC is None:
        _NC = build_module()
    return _NC


def make_in_maps(encoded_spans, SVO_emb, pooled_output, sent2word_adj, aug_adj,
                 punct_graph, w_rel, w_root, b_rgcn, w_lin, att_x, att_e, b_hgcn,
                 ie_w1, ie_b1, ie_w2, ie_b2):
    f = np.float32
    bf = ml_dtypes.bfloat16
    x = np.asarray(encoded_spans, f)                               # [BS, N, C]
    aug = np.asarray(aug_adj, f)
    pun = np.asarray(punct_graph, f)
    A = np.stack([pun * (1.0 - aug), aug], axis=1)                 # [BS, 2, N, N]
    deg = A.sum(axis=2)                                            # in-degree of target j
    adjn = A / np.where(deg > 0, deg, 1.0)[:, :, None, :]
    Hinc = np.concatenate([np.ones((BS, N, 1), f),
                           np.asarray(sent2word_adj, f)], axis=2)  # [BS, N, M]
    hbm = np.where(Hinc > 0, 0.0, -50.0).astype(f)
    Dn = Hinc.sum(axis=2)
    ivdq = (0.25 / np.where(Dn > 0, Dn, 1.0)).astype(f)            # [BS, N]
    Be = Hinc.sum(axis=1)
    ivb = np.where(Be > 0, 1.0 / np.where(Be > 0, Be, 1.0), 0.0).astype(f)  # [BS, M]
    e_attr = np.concatenate([np.asarray(pooled_output, f)[:, None, :],
                             np.asarray(SVO_emb, f)], axis=1)      # [BS, M, C]
    wl = np.ascontiguousarray(np.asarray(w_lin, f))                # [L, C, HH*C]
    wl4 = wl.reshape(L, C, HH, C)
    ux = np.einsum("lchk,lhk->lch", wl4, np.asarray(att_x, f))     # [L, C, HH]
    ue = np.einsum("lchk,lhk->lch", wl4, np.asarray(att_e, f))
    an0 = np.einsum("bnc,ch->bnh", x, ux[0]).astype(f)             # [BS, N, HH]
    ae = np.einsum("bmc,lch->lbhm", e_attr, ue)                    # [L, BS, HH, M]
    wr = np.asarray(w_rel, f)
    wcat = np.concatenate([wr[:, 0], wr[:, 1], np.asarray(w_root, f)], axis=2)
    x0T = np.ascontiguousarray(x.transpose(0, 2, 1))               # [BS, C, N]

    shared = {
        "wlin": wl.astype(bf),
        "wcat": np.ascontiguousarray(wcat).astype(bf),
        "ux1": np.ascontiguousarray(ux[1]).astype(bf),
        "iw1": np.asarray(ie_w1, f).astype(bf),
        "iw2": np.asarray(ie_w2, f).astype(bf),
        "bcolr": np.asarray(b_rgcn, f),
        "bcolh": np.asarray(b_hgcn, f),
        "browr": np.asarray(b_rgcn, f).astype(bf),
        "browh": np.asarray(b_hgcn, f).astype(bf),
        "ib1": np.asarray(ie_b1, f).astype(bf),
        "ib2": np.asarray(ie_b2, f).astype(bf),
        "eyeb": np.eye(128, dtype=f).astype(bf),
        "onesr": np.ones((1, 128), f).astype(bf),
        "ones4": np.ones((1, 4), f).astype(bf),
    }
    in_maps = []
    for c in range(NCORES):
        sl = slice(c * BSL, (c + 1) * BSL)
        m = dict(shared)
        m["x0T"] = np.ascontiguousarray(x0T[sl]).astype(bf)
        m["adjn"] = np.ascontiguousarray(adjn[sl]).astype(bf)
        m["hbm"] = np.ascontiguousarray(hbm[sl]).astype(bf)
        m["ivdq"] = np.ascontiguousarray(ivdq[sl])
        m["ivb"] = np.ascontiguousarray(ivb[sl])[:, :, None]
        m["an0"] = np.ascontiguousarray(an0[sl])
        m["ae"] = np.ascontiguousarray(ae[:, sl]).astype(bf)
        in_maps.append(m)
    return in_maps


def run(in_maps, trace=False, **kw):
    nc = _get_nc()
    return run_bass_kernel_spmd(nc, in_maps, list(range(NCORES)), trace=trace, **kw)


def kernel(**inputs):
    in_maps = make_in_maps(**inputs)
    res = run(in_maps)
    x_r = np.concatenate([np.asarray(res.results[c]["outr"]) for c in range(NCORES)], axis=0)
    x_h = np.concatenate([np.asarray(res.results[c]["outh"]) for c in range(NCORES)], axis=0)
    return x_r.astype(np.float32), x_h.astype(np.float32)


# revision 11
# speedup vs baseline: 1.2147x; 1.1041x over previous
"""Trainium2 Bass kernel for nn_Message_gcn (2-layer RGCN + attention HypergraphConv + info-exchange MLP).

Sharding: pure data parallelism - batch 32 split as 4 samples on each of 8 NeuronCores,
per-layer weights replicated on every core.

v2 layout (PE-continuity focused):
  - graph preprocessing is host-folded: typed adjacency (punct&!aug / aug) is built,
    degree-normalized and shipped as bf16; the hypergraph incidence ships as an additive
    softmax mask (0 / -50), 0.25/D(v) and 1/|e| ship as ready-made columns; layer-0
    node logits a_n and both layers' hyperedge logits a_e (constant across layers)
    are tiny host einsums.
  - per layer, all alpha-independent matmuls (x @ [w_lin | w_rel | w_root]) are emitted
    as one dense per-sample "bulk" block so the PE stays busy (HAM stays at 8/8) while
    DVE/GpSimd/Act run the masked-softmax chains.
  - layer-0 outputs are produced directly in transposed [C, N] layout (the layout layer 1
    consumes): hypergraph out = msg^T @ alpha2^T and RGCN agg = xw^T @ Anorm, with the
    root term applied as an identity-matmul and the bias folded into the PSUM->SBUF
    relu (per-partition Act bias / DVE tensor_scalar).
  - layer-1 outputs are row-major bf16 and stream straight to DRAM; node-0 rows come
    from the info-exchange MLP output.
"""

import sys

sys.path.insert(0, "/opt/trn_rl_repo")

from contextlib import ExitStack

import numpy as np
import ml_dtypes

import concourse.bass as bass
import concourse.tile as tile
from concourse import bacc, mybir
from concourse.bass_utils import run_bass_kernel_spmd

BS, N, E, C, HH, L = 32, 256, 64, 512, 4, 2
M = E + 1
NCORES = 8
BSL = BS // NCORES          # samples per core
NB = N // 128               # node partition tiles
CT = C // 128               # channel partition tiles
C2 = 2 * C
KT2 = C2 // 128             # 2C partition tiles (ie)

f32 = mybir.dt.float32
bf16 = mybir.dt.bfloat16
AF = mybir.ActivationFunctionType
ALU = mybir.AluOpType
AX = mybir.AxisListType


def _ins0(sl: bass.AP, count: int, pos: int) -> bass.AP:
    """Insert a 0-stride (broadcast) dim of `count` into an AP's free dims at
    position `pos` (0 = right after the partition dim, -1 = innermost)."""
    ap = [list(p) for p in sl.ap]
    if pos == -1:
        pos = len(ap) - 1
    ap.insert(1 + pos, [0, count])
    return bass.AP(tensor=sl.tensor, offset=sl.offset, ap=ap)


def _bcast_p(sl: bass.AP, count: int) -> bass.AP:
    """Prepend a 0-stride partition dim of `count` to a DRAM AP (DMA-replicates
    the same source bytes into every partition)."""
    ap = [list(p) for p in sl.ap]
    return bass.AP(tensor=sl.tensor, offset=sl.offset, ap=[[0, count]] + ap)


def build_module():
    nc = bacc.Bacc("TRN2", target_bir_lowering=False, debug=False)

    # ---- DRAM I/O ----
    d_x0T = nc.dram_tensor("x0T", [BSL, C, N], bf16, kind="ExternalInput")
    d_adjn = nc.dram_tensor("adjn", [BSL, 2, N, N], bf16, kind="ExternalInput")
    d_hb = nc.dram_tensor("hbm", [BSL, N, M], bf16, kind="ExternalInput")
    d_ivdq = nc.dram_tensor("ivdq", [BSL, N], f32, kind="ExternalInput")
    d_ivb = nc.dram_tensor("ivb", [BSL, M, 1], f32, kind="ExternalInput")
    d_an0 = nc.dram_tensor("an0", [BSL, N, HH], f32, kind="ExternalInput")
    d_ae = nc.dram_tensor("ae", [L, BSL, HH, M], bf16, kind="ExternalInput")
    d_wlin = nc.dram_tensor("wlin", [L, C, HH * C], bf16, kind="ExternalInput")
    d_wcat = nc.dram_tensor("wcat", [L, C, 3 * C], bf16, kind="ExternalInput")
    d_ux1 = nc.dram_tensor("ux1", [C, HH], bf16, kind="ExternalInput")
    d_iw1 = nc.dram_tensor("iw1", [L, C2, C2], bf16, kind="ExternalInput")
    d_iw2 = nc.dram_tensor("iw2", [L, C2, C2], bf16, kind="ExternalInput")
    d_bcolr = nc.dram_tensor("bcolr", [L, C], f32, kind="ExternalInput")
    d_bcolh = nc.dram_tensor("bcolh", [L, C], f32, kind="ExternalInput")
    d_browr = nc.dram_tensor("browr", [L, C], bf16, kind="ExternalInput")
    d_browh = nc.dram_tensor("browh", [L, C], bf16, kind="ExternalInput")
    d_ib1 = nc.dram_tensor("ib1", [L, C2], bf16, kind="ExternalInput")
    d_ib2 = nc.dram_tensor("ib2", [L, C2], bf16, kind="ExternalInput")
    d_eyeb = nc.dram_tensor("eyeb", [128, 128], bf16, kind="ExternalInput")
    d_ones = nc.dram_tensor("onesr", [1, 128], bf16, kind="ExternalInput")
    d_ones4 = nc.dram_tensor("ones4", [1, 4], bf16, kind="ExternalInput")
    d_outr = nc.dram_tensor("outr", [BSL, N, C], bf16, kind="ExternalOutput")
    d_outh = nc.dram_tensor("outh", [BSL, N, C], bf16, kind="ExternalOutput")

    with ExitStack() as ctx:
        tc = ctx.enter_context(tile.TileContext(nc))
        const = ctx.enter_context(tc.tile_pool(name="const", bufs=1))
        wts = ctx.enter_context(tc.tile_pool(name="wts", bufs=2))
        wie = ctx.enter_context(tc.tile_pool(name="wie", bufs=1))
        xst = ctx.enter_context(tc.tile_pool(name="xst", bufs=8))
        acts = ctx.enter_context(tc.tile_pool(name="acts", bufs=3))
        alph = ctx.enter_context(tc.tile_pool(name="alph", bufs=3))
        msgs = ctx.enter_context(tc.tile_pool(name="msgs", bufs=2))
        outs = ctx.enter_context(tc.tile_pool(name="outs", bufs=2))
        iep = ctx.enter_context(tc.tile_pool(name="iep", bufs=1))
        blk = ctx.enter_context(tc.tile_pool(name="blk", bufs=4, space="PSUM"))
        pst = ctx.enter_context(tc.tile_pool(name="pst", bufs=2, space="PSUM"))
        pmx = ctx.enter_context(tc.tile_pool(name="pmx", bufs=2, space="PSUM"))

        # round-robin over the two PSUM-capable copy engines
        _eng = [0]

        def cpeng():
            _eng[0] ^= 1
            return nc.vector if _eng[0] else nc.scalar

        def relu_bias(dst, src, bias_ap):
            e = cpeng()
            if e is nc.scalar:
                e.activation(dst, src, AF.Relu, bias=bias_ap)
            else:
                e.tensor_scalar(dst, src, bias_ap, 0.0, op0=ALU.add, op1=ALU.max)

        def copy(dst, src, scale=None):
            e = cpeng()
            if e is nc.scalar:
                if scale is None:
                    e.copy(dst, src)
                else:
                    e.activation(dst, src, AF.Copy, scale=scale)
            else:
                if scale is None:
                    e.tensor_copy(dst, src)
                else:
                    e.tensor_scalar(dst, src, scale, None, op0=ALU.mult)

        # ---- input / constant DMAs ----
        # sync queue carries the bulk-critical stream in consumption order:
        # sample-0 inputs + first weight chunks first, so the PE can start
        # within a few us of kernel start.
        x0Ts = [None] * BSL
        wlins = [None] * L
        wcats = [None] * L
        for s in range(BSL):
            x0Ts[s] = xst.tile([128, CT, N], bf16, tag="xst", name=f"x0T_{s}")
        wlins[0] = wts.tile([128, CT, HH * C], bf16, tag="wlin", name="wlin0")
        wcats[0] = wts.tile([128, CT, 3 * C], bf16, tag="wcat", name="wcat0", bufs=1)
        dw = d_wlin[0].rearrange("(ct p) k -> p ct k", p=128)
        dc = d_wcat[0].rearrange("(ct p) k -> p ct k", p=128)
        nc.sync.dma_start(x0Ts[0][:], d_x0T[0].rearrange("(ct p) n -> p ct n", p=128))
        for k in range(7):
            if k < HH:
                for ct in range(CT):
                    nc.sync.dma_start(wlins[0][:, ct, k * C : (k + 1) * C],
                                      dw[:, ct, k * C : (k + 1) * C])
            else:
                r3 = k - 4
                for ct in range(CT):
                    nc.sync.dma_start(wcats[0][:, ct, r3 * C : (r3 + 1) * C],
                                      dc[:, ct, r3 * C : (r3 + 1) * C])
            if k < 3:
                nc.sync.dma_start(x0Ts[k + 1][:],
                                  d_x0T[k + 1].rearrange("(ct p) n -> p ct n", p=128))

        identb = const.tile([128, 128], bf16)
        nc.sync.dma_start(identb[:], d_eyeb[:])
        ones_row = const.tile([1, 128], bf16)
        nc.sync.dma_start(ones_row[:], d_ones[:])
        ones4 = const.tile([1, 4], bf16)
        nc.sync.dma_start(ones4[:], d_ones4[:])
        ux1 = const.tile([128, CT, HH], bf16)
        nc.sync.dma_start(ux1[:], d_ux1.rearrange("(ct p) h -> p ct h", p=128))

        an0s, abs_, hbs, ivdqs, ivbs, Afns = [], [[], []], [], [], [], []
        for s in range(BSL):
            t = const.tile([128, NB, HH], f32, name=f"an0_{s}")
            nc.scalar.dma_start(t[:], d_an0[s].rearrange("(t p) h -> p t h", p=128))
            an0s.append(t)
            for l in range(L):
                a = const.tile([128, HH, M], bf16, name=f"ab{l}_{s}")
                nc.scalar.dma_start(a[:], _bcast_p(d_ae[l, s], 128))
                abs_[l].append(a)
            t = const.tile([128, NB, M], bf16, name=f"hb_{s}")
            nc.scalar.dma_start(t[:], d_hb[s].rearrange("(t p) m -> p t m", p=128))
            hbs.append(t)
            t = const.tile([128, NB], f32, name=f"ivdq_{s}")
            nc.scalar.dma_start(t[:], d_ivdq[s].rearrange("(t p) -> p t", p=128))
            ivdqs.append(t)
            t = const.tile([M, 1], f32, name=f"ivb_{s}")
            nc.scalar.dma_start(t[:], d_ivb[s])
            ivbs.append(t)
            t = const.tile([128, 2, NB, N], bf16, name=f"afn_{s}")
            nc.gpsimd.dma_start(t[:], d_adjn[s].rearrange("r (t p) j -> p r t j", p=128))
            Afns.append(t)

        bcolr, bcolh, browr, browh, ib1r, ib2r = [], [], [], [], [], []
        for l in range(L):
            t = const.tile([128, CT], f32, name=f"bcolr{l}")
            nc.scalar.dma_start(t[:], d_bcolr[l].rearrange("(ct p) -> p ct", p=128))
            bcolr.append(t)
            t = const.tile([128, CT], f32, name=f"bcolh{l}")
            nc.scalar.dma_start(t[:], d_bcolh[l].rearrange("(ct p) -> p ct", p=128))
            bcolh.append(t)
            t = const.tile([1, C], bf16, name=f"browr{l}")
            nc.scalar.dma_start(t[:], d_browr[l : l + 1, :])
            browr.append(t)
            t = const.tile([1, C], bf16, name=f"browh{l}")
            nc.scalar.dma_start(t[:], d_browh[l : l + 1, :])
            browh.append(t)
            t = const.tile([1, C2], bf16, name=f"ib1_{l}")
            nc.scalar.dma_start(t[:], d_ib1[l : l + 1, :])
            ib1r.append(t)
            t = const.tile([1, C2], bf16, name=f"ib2_{l}")
            nc.scalar.dma_start(t[:], d_ib2[l : l + 1, :])
            ib2r.append(t)

        # layer-1 weights and IE weights stream in behind the critical path
        wlins[1] = wts.tile([128, CT, HH * C], bf16, tag="wlin", name="wlin1")
        dw = d_wlin[1].rearrange("(ct p) k -> p ct k", p=128)
        for h in range(HH):
            nc.gpsimd.dma_start(wlins[1][:, :, h * C : (h + 1) * C], dw[:, :, h * C : (h + 1) * C])
        iw1_t = wie.tile([128, KT2, C2], bf16, tag="iw1")
        nc.sync.dma_start(iw1_t[:], d_iw1[0].rearrange("(kt p) k -> p kt k", p=128))
        iw2_t = wie.tile([128, KT2, C2], bf16, tag="iw2")
        nc.sync.dma_start(iw2_t[:], d_iw2[0].rearrange("(kt p) k -> p kt k", p=128))
        wcats[1] = wts.tile([128, CT, 3 * C], bf16, tag="wcat", name="wcat1", bufs=1)
        dc1 = d_wcat[1].rearrange("(ct p) k -> p ct k", p=128)
        for r3 in range(3):
            nc.sync.dma_start(wcats[1][:, :, r3 * C : (r3 + 1) * C], dc1[:, :, r3 * C : (r3 + 1) * C])

        # ---- per-(layer, sample) pieces ----
        xrTs = list(x0Ts)
        xhTs = list(x0Ts)
        alphas = [None] * BSL
        a2bs = [None] * BSL
        xls = [None] * BSL
        xws = [None] * BSL
        xwrs = [None] * BSL
        an1s = [None] * BSL
        ctxT = None

        def alpha_chain(l, s):
            """Masked softmax over incident hyperedges -> alpha (for msg) and
            a2b = alpha * 0.25/D(v) (for the node-side aggregation).
            GpSimd builds the logits, Act does the exp, DVE the reductions."""
            an_src = an0s[s] if l == 0 else an1s[s]
            t1 = alph.tile([128, NB, HH, M], f32, tag="t1")
            nc.gpsimd.tensor_tensor(t1[:], _ins0(abs_[l][s][:], NB, 0),
                                    _ins0(an_src[:], M, -1), op=ALU.add)
            nc.vector.scalar_tensor_tensor(t1[:], t1[:], 0.2, t1[:],
                                           op0=ALU.mult, op1=ALU.max)
            nc.gpsimd.tensor_tensor(t1[:], t1[:], _ins0(hbs[s][:], HH, 1), op=ALU.add)
            nmax = alph.tile([128, NB, HH], f32, tag="nmax")
            nc.vector.tensor_reduce(nmax[:], t1[:], axis=AX.X, op=ALU.max, negate=True)
            nc.gpsimd.tensor_tensor(t1[:], t1[:], _ins0(nmax[:], M, -1), op=ALU.add)
            nc.scalar.activation(t1[:], t1[:], AF.Exp)
            ssum = alph.tile([128, NB, HH], f32, tag="ssum")
            nc.vector.tensor_reduce(ssum[:], t1[:], axis=AX.X, op=ALU.add)
            rs = alph.tile([128, NB, HH], f32, tag="rs")
            nc.vector.reciprocal(rs[:], ssum[:])
            rcol2 = alph.tile([128, NB, HH], f32, tag="rcol2")
            nc.vector.tensor_tensor(rcol2[:], rs[:], _ins0(ivdqs[s][:], HH, -1), op=ALU.mult)
            al = alph.tile([128, NB, HH, M], bf16, tag="alpha")
            nc.vector.tensor_tensor(al[:], t1[:], _ins0(rs[:], M, -1), op=ALU.mult)
            a2 = alph.tile([128, NB, HH, M], bf16, tag="a2b")
            nc.vector.tensor_tensor(a2[:], t1[:], _ins0(rcol2[:], M, -1), op=ALU.mult)
            alphas[s] = al
            a2bs[s] = a2

        def bulk(l, s):
            """x @ [w_lin | w_rel0 | w_rel1 | w_root]: all alpha-independent
            PE work for (l, s), drained chunk-by-chunk into bf16 SBUF."""
            xl = acts.tile([128, HH, NB, C], bf16, tag="xl")
            xw = acts.tile([128, NB, 2, C], bf16, tag="xw")
            xwr = acts.tile([128, NB, C], bf16, tag="xwr")
            for k in range(7):
                for nt in range(NB):
                    if k < HH:
                        stat, wt, wsl = xhTs[s], wlins[l], slice(k * C, (k + 1) * C)
                        dst = xl[:, k, nt, :]
                    else:
                        stat, wt, wsl = xrTs[s], wcats[l], slice((k - 4) * C, (k - 3) * C)
                        dst = xw[:, nt, k - 4, :] if k < 6 else xwr[:, nt, :]
                    ps = blk.tile([128, C], f32, tag="blk")
                    for ct in range(CT):
                        nc.tensor.matmul(ps[:], stat[:, ct, nt * 128 : (nt + 1) * 128],
                                         wt[:, ct, wsl], start=(ct == 0), stop=(ct == CT - 1))
                    copy(dst, ps[:])
            xls[s], xws[s], xwrs[s] = xl, xw, xwr

        def phase2(l, s):
            """alpha-dependent per-sample work: alpha transposes, msg,
            hypergraph out, RGCN aggregation, ctx column."""
            al, a2 = alphas[s], a2bs[s]
            xl, xw, xwr = xls[s], xws[s], xwrs[s]
            Afn = Afns[s]

            a3T = msgs.tile([M, HH, N], bf16, tag="a3T")
            for nb in range(NB):
                for h in range(HH):
                    tp = pmx.tile([M, 128], bf16, tag="mx", padded_shape=[M, 1024])
                    nc.tensor.transpose(tp[:], a2[:, nb, h, :], identb[:])
                    copy(a3T[:, h, nb * 128 : (nb + 1) * 128], tp[:])

            msg = msgs.tile([M, HH, C], bf16, tag="msg")
            for h in range(HH):
                mp = blk.tile([M, C], f32, tag="blk")
                for nb in range(NB):
                    nc.tensor.matmul(mp[:], al[:, nb, h, :], xl[:, h, nb, :],
                                     start=(nb == 0), stop=(nb == NB - 1))
                copy(msg[:, h, :], mp[:], scale=ivbs[s][:, 0:1])

            if l == 0:
                # transposed outputs -> next-layer state tiles [C, N]
                xhT1 = xst.tile([128, CT, N], bf16, tag="xst")
                xrT1 = xst.tile([128, CT, N], bf16, tag="xst")
                for g in range(CT // 2):
                    ph = pst.tile([128, 2, N], f32, tag="st")
                    for j in range(2):
                        ct = 2 * g + j
                        for h in range(HH):
                            nc.tensor.matmul(ph[:, j, :], msg[:, h, ct * 128 : (ct + 1) * 128],
                                             a3T[:, h, :], start=(h == 0), stop=(h == HH - 1))
                    for j in range(2):
                        ct = 2 * g + j
                        relu_bias(xhT1[:, ct, :], ph[:, j, :], bcolh[l][:, ct : ct + 1])
                for g in range(CT // 2):
                    pr = pst.tile([128, 2, N], f32, tag="st")
                    for j in range(2):
                        dt = 2 * g + j
                        first = True
                        for r in range(2):
                            for it in range(NB):
                                nc.tensor.matmul(pr[:, j, :],
                                                 xw[:, it, r, dt * 128 : (dt + 1) * 128],
                                                 Afn[:, r, it, :], start=first, stop=False)
                                first = False
                        for it in range(NB):
                            nc.tensor.matmul(pr[:, j, it * 128 : (it + 1) * 128],
                                             xwr[:, it, dt * 128 : (dt + 1) * 128],
                                             identb[:], start=False, stop=(it == NB - 1))
                    for j in range(2):
                        dt = 2 * g + j
                        relu_bias(xrT1[:, dt, :], pr[:, j, :], bcolr[l][:, dt : dt + 1])
                xhTs[s] = xhT1
                xrTs[s] = xrT1
                # ctx columns (pre-patch node-0 output)
                nc.vector.tensor_copy(ctxT[:, 0:CT, s : s + 1], xrT1[:, :, 0:1])
                nc.vector.tensor_copy(ctxT[:, CT : 2 * CT, s : s + 1], xhT1[:, :, 0:1])
            else:
                # row-major bf16 outputs, streamed to DRAM (rows 1..N-1)
                outh_t = outs.tile([128, NB, C], bf16, tag="outh")
                for nb in range(NB):
                    po = blk.tile([128, C], f32, tag="blk")
                    for h in range(HH):
                        nc.tensor.matmul(po[:], a3T[:, h, nb * 128 : (nb + 1) * 128],
                                         msg[:, h, :], start=(h == 0), stop=False)
                    nc.tensor.matmul(po[:], ones_row[:], browh[l][:], start=False, stop=True)
                    e = cpeng()
                    if e is nc.scalar:
                        e.activation(outh_t[:, nb, :], po[:], AF.Relu)
                    else:
                        e.tensor_scalar(outh_t[:, nb, :], po[:], 0.0, None, op0=ALU.max)
                outr_t = outs.tile([128, NB, C], bf16, tag="outr")
                for jb in range(NB):
                    po = blk.tile([128, C], f32, tag="blk")
                    first = True
                    for r in range(2):
                        for it in range(NB):
                            nc.tensor.matmul(po[:], Afn[:, r, it, jb * 128 : (jb + 1) * 128],
                                             xw[:, it, r, :], start=first, stop=False)
                            first = False
                    nc.tensor.matmul(po[:], identb[:], xwr[:, jb, :], start=False, stop=False)
                    nc.tensor.matmul(po[:], ones_row[:], browr[l][:], start=False, stop=True)
                    e = cpeng()
                    if e is nc.scalar:
                        e.activation(outr_t[:, jb, :], po[:], AF.Relu)
                    else:
                        e.tensor_scalar(outr_t[:, jb, :], po[:], 0.0, None, op0=ALU.max)
                # ctx rows -> columns
                cps = pmx.tile([128, 2, CT, 2], bf16, tag="mx", padded_shape=[128, 2, CT, 64])
                for ct in range(CT):
                    nc.tensor.transpose(cps[:, 0, ct, 0:1],
                                        outr_t[0:1, 0, ct * 128 : (ct + 1) * 128],
                                        identb[0:1, 0:1])
                    nc.tensor.transpose(cps[:, 1, ct, 0:1],
                                        outh_t[0:1, 0, ct * 128 : (ct + 1) * 128],
                                        identb[0:1, 0:1])
                nc.vector.tensor_copy(
                    ctxT[:, :, s : s + 1].rearrange("p (r ct) o -> p r (ct o)", r=2),
                    cps[:, :, :, 0])
                for tsrc, dram in ((outr_t, d_outr), (outh_t, d_outh)):
                    nc.sync.dma_start(dram[s, 1:128, :], tsrc[1:128, 0, :])
                    nc.sync.dma_start(dram[s, 128:N, :], tsrc[:, 1, :])

        def ie(l):
            """info-exchange MLP over the BSL ctx columns; layer 0 patches the
            state tiles' node-0 column, layer 1 DMAs the node-0 output rows."""
            y1 = iep.tile([BSL, C2], bf16, tag="y1")
            for ch in range(2):
                yp = blk.tile([BSL, C], f32, tag="blk")
                for kt in range(KT2):
                    nc.tensor.matmul(yp[:], ctxT[:, kt, :], iw1_t[:, kt, ch * C : (ch + 1) * C],
                                     start=(kt == 0), stop=False)
                nc.tensor.matmul(yp[:], ones4[:], ib1r[l][:, ch * C : (ch + 1) * C],
                                 start=False, stop=True)
                e = cpeng()
                if e is nc.scalar:
                    e.activation(y1[:, ch * C : (ch + 1) * C], yp[:], AF.Relu)
                else:
                    e.tensor_scalar(y1[:, ch * C : (ch + 1) * C], yp[:], 0.0, None, op0=ALU.max)
            c2T = iep.tile([128, KT2, BSL], bf16, tag="c2T")
            for kt in range(KT2):
                tp = pmx.tile([128, BSL], bf16, tag="mx", padded_shape=[128, 1024])
                nc.tensor.transpose(tp[:], y1[:, kt * 128 : (kt + 1) * 128], identb[0:BSL, 0:BSL])
                copy(c2T[:, kt, :], tp[:])
            y2 = iep.tile([BSL, C2], bf16, tag="y2")
            for ch in range(2):
                yp = blk.tile([BSL, C], f32, tag="blk")
                for kt in range(KT2):
                    nc.tensor.matmul(yp[:], c2T[:, kt, :], iw2_t[:, kt, ch * C : (ch + 1) * C],
                                     start=(kt == 0), stop=False)
                nc.tensor.matmul(yp[:], ones4[:], ib2r[l][:, ch * C : (ch + 1) * C],
                                 start=False, stop=True)
                copy(y2[:, ch * C : (ch + 1) * C], yp[:])
            if l == 0:
                for kt in range(KT2):
                    tp = pmx.tile([128, BSL], bf16, tag="mx", padded_shape=[128, 1024])
                    nc.tensor.transpose(tp[:], y2[:, kt * 128 : (kt + 1) * 128],
                                        identb[0:BSL, 0:BSL])
                    for s in range(BSL):
                        dst = xrTs[s] if kt < CT else xhTs[s]
                        e = cpeng()
                        if e is nc.scalar:
                            e.copy(dst[:, kt % CT, 0:1], tp[:, s : s + 1])
                        else:
                            e.tensor_copy(dst[:, kt % CT, 0:1], tp[:, s : s + 1])
            else:
                for s in range(BSL):
                    nc.sync.dma_start(d_outr[s, 0:1, :], y2[s : s + 1, 0:C])
                    nc.sync.dma_start(d_outh[s, 0:1, :], y2[s : s + 1, C:C2])

        # ================= layer 0 =================
        ctxT = iep.tile([128, 2 * CT, BSL], bf16, tag="ctxT")
        for s in range(BSL):
            alpha_chain(0, s)
        bulk(0, 0)
        bulk(0, 1)
        phase2(0, 0)
        bulk(0, 2)
        phase2(0, 1)
        bulk(0, 3)
        phase2(0, 2)
        phase2(0, 3)
        ie(0)

        # layer-1 IE weights reuse the same SBUF slots (gated on ie(0) readers)
        iw1_t = wie.tile([128, KT2, C2], bf16, tag="iw1")
        nc.gpsimd.dma_start(iw1_t[:], d_iw1[1].rearrange("(kt p) k -> p kt k", p=128))
        iw2_t = wie.tile([128, KT2, C2], bf16, tag="iw2")
        nc.sync.dma_start(iw2_t[:], d_iw2[1].rearrange("(kt p) k -> p kt k", p=128))

        # ================= layer 1 =================
        ctxT = iep.tile([128, 2 * CT, BSL], bf16, tag="ctxT2")
        for s in range(BSL):
            ap = blk.tile([HH, N], f32, tag="blk", padded_shape=[HH, 512])
            for ct in range(CT):
                nc.tensor.matmul(ap[:], ux1[:, ct, :], xhTs[s][:, ct, :],
                                 start=(ct == 0), stop=(ct == CT - 1))
            anrow = iep.tile([HH, N], bf16, tag="anrow", name=f"anrow_{s}")
            copy(anrow[:], ap[:])
            an1 = alph.tile([128, NB, HH], f32, tag="an1")
            for nb in range(NB):
                tp = pmx.tile([128, HH], bf16, tag="mx", padded_shape=[128, 1024])
                nc.tensor.transpose(tp[:], anrow[:, nb * 128 : (nb + 1) * 128],
                                    identb[0:HH, 0:HH])
                copy(an1[:, nb, :], tp[:])
            an1s[s] = an1
        for s in range(BSL):
            alpha_chain(1, s)
        bulk(1, 0)
        bulk(1, 1)
        phase2(1, 0)
        bulk(1, 2)
        phase2(1, 1)
        bulk(1, 3)
        phase2(1, 2)
        phase2(1, 3)
        ie(1)

    nc.compile()
    return nc


_NC = None


def _get_nc():
    global _NC
    if _NC is None:
        _NC = build_module()
    return _NC


def make_in_maps(encoded_spans, SVO_emb, pooled_output, sent2word_adj, aug_adj,
                 punct_graph, w_rel, w_root, b_rgcn, w_lin, att_x, att_e, b_hgcn,
                 ie_w1, ie_b1, ie_w2, ie_b2):
    f = np.float32
    bf = ml_dtypes.bfloat16
    x = np.asarray(encoded_spans, f)                               # [BS, N, C]
    aug = np.asarray(aug_adj, f)
    pun = np.asarray(punct_graph, f)
    A = np.stack([pun * (1.0 - aug), aug], axis=1)                 # [BS, 2, N, N]
    deg = A.sum(axis=2)                                            # in-degree of target j
    adjn = A / np.where(deg > 0, deg, 1.0)[:, :, None, :]
    Hinc = np.concatenate([np.ones((BS, N, 1), f),
                           np.asarray(sent2word_adj, f)], axis=2)  # [BS, N, M]
    hbm = np.where(Hinc > 0, 0.0, -50.0).astype(f)
    Dn = Hinc.sum(axis=2)
    ivdq = (0.25 / np.where(Dn > 0, Dn, 1.0)).astype(f)            # [BS, N]
    Be = Hinc.sum(axis=1)
    ivb = np.where(Be > 0, 1.0 / np.where(Be > 0, Be, 1.0), 0.0).astype(f)  # [BS, M]
    e_attr = np.concatenate([np.asarray(pooled_output, f)[:, None, :],
                             np.asarray(SVO_emb, f)], axis=1)      # [BS, M, C]
    wl = np.ascontiguousarray(np.asarray(w_lin, f))                # [L, C, HH*C]
    wl4 = wl.reshape(L, C, HH, C)
    ux = np.einsum("lchk,lhk->lch", wl4, np.asarray(att_x, f))     # [L, C, HH]
    ue = np.einsum("lchk,lhk->lch", wl4, np.asarray(att_e, f))
    an0 = np.einsum("bnc,ch->bnh", x, ux[0]).astype(f)             # [BS, N, HH]
    ae = np.einsum("bmc,lch->lbhm", e_attr, ue)                    # [L, BS, HH, M]
    wr = np.asarray(w_rel, f)
    wcat = np.concatenate([wr[:, 0], wr[:, 1], np.asarray(w_root, f)], axis=2)
    x0T = np.ascontiguousarray(x.transpose(0, 2, 1))               # [BS, C, N]

    shared = {
        "wlin": wl.astype(bf),
        "wcat": np.ascontiguousarray(wcat).astype(bf),
        "ux1": np.ascontiguousarray(ux[1]).astype(bf),
        "iw1": np.asarray(ie_w1, f).astype(bf),
        "iw2": np.asarray(ie_w2, f).astype(bf),
        "bcolr": np.asarray(b_rgcn, f),
        "bcolh": np.asarray(b_hgcn, f),
        "browr": np.asarray(b_rgcn, f).astype(bf),
        "browh": np.asarray(b_hgcn, f).astype(bf),
        "ib1": np.asarray(ie_b1, f).astype(bf),
        "ib2": np.asarray(ie_b2, f).astype(bf),
        "eyeb": np.eye(128, dtype=f).astype(bf),
        "onesr": np.ones((1, 128), f).astype(bf),
        "ones4": np.ones((1, 4), f).astype(bf),
    }
    in_maps = []
    for c in range(NCORES):
        sl = slice(c * BSL, (c + 1) * BSL)
        m = dict(shared)
        m["x0T"] = np.ascontiguousarray(x0T[sl]).astype(bf)
        m["adjn"] = np.ascontiguousarray(adjn[sl]).astype(bf)
        m["hbm"] = np.ascontiguousarray(hbm[sl]).astype(bf)
        m["ivdq"] = np.ascontiguousarray(ivdq[sl])
        m["ivb"] = np.ascontiguousarray(ivb[sl])[:, :, None]
        m["an0"] = np.ascontiguousarray(an0[sl])
        m["ae"] = np.ascontiguousarray(ae[:, sl]).astype(bf)
        in_maps.append(m)
    return in_maps


def run(in_maps, trace=False, **kw):
    nc = _get_nc()
    return run_bass_kernel_spmd(nc, in_maps, list(range(NCORES)), trace=trace, **kw)


def kernel(**inputs):
    in_maps = make_in_maps(**inputs)
    res = run(in_maps)
    x_r = np.concatenate([np.asarray(res.results[c]["outr"]) for c in range(NCORES)], axis=0)
    x_h = np.concatenate([np.asarray(res.results[c]["outh"]) for c in range(NCORES)], axis=0)
    return x_r.astype(np.float32), x_h.astype(np.float32)


# revision 14
# speedup vs baseline: 1.2675x; 1.0435x over previous
"""Trainium2 Bass kernel for nn_Message_gcn (2-layer RGCN + attention HypergraphConv + info-exchange MLP).

Sharding: pure data parallelism - batch 32 split as 4 samples on each of 8 NeuronCores,
per-layer weights replicated on every core.

v2 layout (PE-continuity focused):
  - graph preprocessing is host-folded: typed adjacency (punct&!aug / aug) is built,
    degree-normalized and shipped as bf16; the hypergraph incidence ships as an additive
    softmax mask (0 / -50), 0.25/D(v) and 1/|e| ship as ready-made columns; layer-0
    node logits a_n and both layers' hyperedge logits a_e (constant across layers)
    are tiny host einsums.
  - per layer, all alpha-independent matmuls (x @ [w_lin | w_rel | w_root]) are emitted
    as one dense per-sample "bulk" block so the PE stays busy (HAM stays at 8/8) while
    DVE/GpSimd/Act run the masked-softmax chains.
  - layer-0 outputs are produced directly in transposed [C, N] layout (the layout layer 1
    consumes): hypergraph out = msg^T @ alpha2^T and RGCN agg = xw^T @ Anorm, with the
    root term applied as an identity-matmul and the bias folded into the PSUM->SBUF
    relu (per-partition Act bias / DVE tensor_scalar).
  - layer-1 outputs are row-major bf16 and stream straight to DRAM; node-0 rows come
    from the info-exchange MLP output.
"""

import sys

sys.path.insert(0, "/opt/trn_rl_repo")

from contextlib import ExitStack

import numpy as np
import ml_dtypes

import concourse.bass as bass
import concourse.tile as tile
from concourse import bacc, mybir
from concourse.bass_utils import run_bass_kernel_spmd

BS, N, E, C, HH, L = 32, 256, 64, 512, 4, 2
M = E + 1
NCORES = 8
BSL = BS // NCORES          # samples per core
NB = N // 128               # node partition tiles
CT = C // 128               # channel partition tiles
C2 = 2 * C
KT2 = C2 // 128             # 2C partition tiles (ie)

f32 = mybir.dt.float32
bf16 = mybir.dt.bfloat16
AF = mybir.ActivationFunctionType
ALU = mybir.AluOpType
AX = mybir.AxisListType


def _ins0(sl: bass.AP, count: int, pos: int) -> bass.AP:
    """Insert a 0-stride (broadcast) dim of `count` into an AP's free dims at
    position `pos` (0 = right after the partition dim, -1 = innermost)."""
    ap = [list(p) for p in sl.ap]
    if pos == -1:
        pos = len(ap) - 1
    ap.insert(1 + pos, [0, count])
    return bass.AP(tensor=sl.tensor, offset=sl.offset, ap=ap)


def _bcast_p(sl: bass.AP, count: int) -> bass.AP:
    """Prepend a 0-stride partition dim of `count` to a DRAM AP (DMA-replicates
    the same source bytes into every partition)."""
    ap = [list(p) for p in sl.ap]
    return bass.AP(tensor=sl.tensor, offset=sl.offset, ap=[[0, count]] + ap)


def build_module():
    nc = bacc.Bacc("TRN2", target_bir_lowering=False, debug=False)

    # ---- DRAM I/O ----
    # all large inputs ship partition-major from the host so every DMA
    # descriptor is >=2KB contiguous per partition
    d_x0T = nc.dram_tensor("x0T", [BSL, 128, CT, N], bf16, kind="ExternalInput")
    d_adjn = nc.dram_tensor("adjn", [BSL, 128, 2, NB, N], bf16, kind="ExternalInput")
    d_hb = nc.dram_tensor("hbm", [BSL, 128, NB, M], bf16, kind="ExternalInput")
    d_ivdq = nc.dram_tensor("ivdq", [BSL, 128, NB], f32, kind="ExternalInput")
    d_ivb = nc.dram_tensor("ivb", [BSL, M, 1], f32, kind="ExternalInput")
    d_an0 = nc.dram_tensor("an0", [BSL, 128, NB, HH], f32, kind="ExternalInput")
    d_ae = nc.dram_tensor("ae", [L, BSL, HH, M], bf16, kind="ExternalInput")
    d_w = nc.dram_tensor("wcomb", [L, 128, 7, CT, 512], bf16, kind="ExternalInput")
    d_ux1 = nc.dram_tensor("ux1", [128, CT, HH], bf16, kind="ExternalInput")
    d_iw1 = nc.dram_tensor("iw1", [L, 128, KT2, C2], bf16, kind="ExternalInput")
    d_iw2 = nc.dram_tensor("iw2", [L, 128, KT2, C2], bf16, kind="ExternalInput")
    d_bcolr = nc.dram_tensor("bcolr", [L, 128, CT], f32, kind="ExternalInput")
    d_bcolh = nc.dram_tensor("bcolh", [L, 128, CT], f32, kind="ExternalInput")
    d_browr = nc.dram_tensor("browr", [L, C], bf16, kind="ExternalInput")
    d_browh = nc.dram_tensor("browh", [L, C], bf16, kind="ExternalInput")
    d_ib1 = nc.dram_tensor("ib1", [L, C2], bf16, kind="ExternalInput")
    d_ib2 = nc.dram_tensor("ib2", [L, C2], bf16, kind="ExternalInput")
    d_eyeb = nc.dram_tensor("eyeb", [128, 128], bf16, kind="ExternalInput")
    d_ones = nc.dram_tensor("onesr", [1, 128], bf16, kind="ExternalInput")
    d_ones4 = nc.dram_tensor("ones4", [1, 4], bf16, kind="ExternalInput")
    d_outr = nc.dram_tensor("outr", [BSL, N, C], bf16, kind="ExternalOutput")
    d_outh = nc.dram_tensor("outh", [BSL, N, C], bf16, kind="ExternalOutput")

    with ExitStack() as ctx:
        tc = ctx.enter_context(tile.TileContext(nc))
        const = ctx.enter_context(tc.tile_pool(name="const", bufs=1))
        wts = ctx.enter_context(tc.tile_pool(name="wts", bufs=2))
        wie = ctx.enter_context(tc.tile_pool(name="wie", bufs=1))
        xst = ctx.enter_context(tc.tile_pool(name="xst", bufs=8))
        acts = ctx.enter_context(tc.tile_pool(name="acts", bufs=3))
        alph = ctx.enter_context(tc.tile_pool(name="alph", bufs=3))
        msgs = ctx.enter_context(tc.tile_pool(name="msgs", bufs=2))
        outs = ctx.enter_context(tc.tile_pool(name="outs", bufs=2))
        iep = ctx.enter_context(tc.tile_pool(name="iep", bufs=1))
        blk = ctx.enter_context(tc.tile_pool(name="blk", bufs=4, space="PSUM"))
        pst = ctx.enter_context(tc.tile_pool(name="pst", bufs=2, space="PSUM"))
        pmx = ctx.enter_context(tc.tile_pool(name="pmx", bufs=2, space="PSUM"))

        # round-robin over the two PSUM-capable copy engines
        _eng = [0]

        def cpeng():
            _eng[0] ^= 1
            return nc.vector if _eng[0] else nc.scalar

        def relu_bias(dst, src, bias_ap):
            e = cpeng()
            if e is nc.scalar:
                e.activation(dst, src, AF.Relu, bias=bias_ap)
            else:
                e.tensor_scalar(dst, src, bias_ap, 0.0, op0=ALU.add, op1=ALU.max)

        def copy(dst, src, scale=None):
            e = cpeng()
            if e is nc.scalar:
                if scale is None:
                    e.copy(dst, src)
                else:
                    e.activation(dst, src, AF.Copy, scale=scale)
            else:
                if scale is None:
                    e.tensor_copy(dst, src)
                else:
                    e.tensor_scalar(dst, src, scale, None, op0=ALU.mult)

        # ---- input / constant DMAs ----
        # sync queue carries the bulk-critical stream in consumption order:
        # sample-0 inputs + first weight chunks first, so the PE can start
        # within a few us of kernel start.
        x0Ts = [None] * BSL
        ws = [None] * L
        for s in range(BSL):
            x0Ts[s] = xst.tile([128, CT, N], bf16, tag="xst", name=f"x0T_{s}")
        ws[0] = wts.tile([128, 7, CT, 512], bf16, tag="w", name="w0")
        nc.sync.dma_start(x0Ts[0][:], d_x0T[0])
        for k in range(7):
            nc.sync.dma_start(ws[0][:, k, :, :], d_w[0, :, k, :, :])
            if k < 3:
                nc.sync.dma_start(x0Ts[k + 1][:], d_x0T[k + 1])

        identb = const.tile([128, 128], bf16)
        nc.sync.dma_start(identb[:], d_eyeb[:])
        ones_row = const.tile([1, 128], bf16)
        nc.sync.dma_start(ones_row[:], d_ones[:])
        ones4 = const.tile([1, 4], bf16)
        nc.sync.dma_start(ones4[:], d_ones4[:])
        ux1 = const.tile([128, CT, HH], bf16)
        nc.sync.dma_start(ux1[:], d_ux1[:])

        an0s, abs_, hbs, ivdqs, ivbs, Afns = [], [[], []], [], [], [], []
        for s in range(BSL):
            t = const.tile([128, NB, HH], f32, name=f"an0_{s}")
            nc.scalar.dma_start(t[:], d_an0[s])
            an0s.append(t)
            for l in range(L):
                a = const.tile([128, HH, M], bf16, name=f"ab{l}_{s}")
                nc.scalar.dma_start(a[:], _bcast_p(d_ae[l, s], 128))
                abs_[l].append(a)
            t = const.tile([128, NB, M], bf16, name=f"hb_{s}")
            nc.scalar.dma_start(t[:], d_hb[s])
            hbs.append(t)
            t = const.tile([128, NB], f32, name=f"ivdq_{s}")
            nc.scalar.dma_start(t[:], d_ivdq[s])
            ivdqs.append(t)
            t = const.tile([M, 1], f32, name=f"ivb_{s}")
            nc.scalar.dma_start(t[:], d_ivb[s])
            ivbs.append(t)
            t = const.tile([128, 2, NB, N], bf16, name=f"afn_{s}")
            nc.gpsimd.dma_start(t[:], d_adjn[s])
            Afns.append(t)

        bcolr, bcolh, browr, browh, ib1r, ib2r = [], [], [], [], [], []
        for l in range(L):
            t = const.tile([128, CT], f32, name=f"bcolr{l}")
            nc.scalar.dma_start(t[:], d_bcolr[l])
            bcolr.append(t)
            t = const.tile([128, CT], f32, name=f"bcolh{l}")
            nc.scalar.dma_start(t[:], d_bcolh[l])
            bcolh.append(t)
            t = const.tile([1, C], bf16, name=f"browr{l}")
            nc.scalar.dma_start(t[:], d_browr[l : l + 1, :])
            browr.append(t)
            t = const.tile([1, C], bf16, name=f"browh{l}")
            nc.scalar.dma_start(t[:], d_browh[l : l + 1, :])
            browh.append(t)
            t = const.tile([1, C2], bf16, name=f"ib1_{l}")
            nc.scalar.dma_start(t[:], d_ib1[l : l + 1, :])
            ib1r.append(t)
            t = const.tile([1, C2], bf16, name=f"ib2_{l}")
            nc.scalar.dma_start(t[:], d_ib2[l : l + 1, :])
            ib2r.append(t)

        # layer-1 weights and IE weights stream in behind the critical path
        ws[1] = wts.tile([128, 7, CT, 512], bf16, tag="w", name="w1")
        for k in range(7):
            nc.gpsimd.dma_start(ws[1][:, k, :, :], d_w[1, :, k, :, :])
        iw1_t = wie.tile([128, KT2, C2], bf16, tag="iw1")
        nc.sync.dma_start(iw1_t[:], d_iw1[0])
        iw2_t = wie.tile([128, KT2, C2], bf16, tag="iw2")
        nc.sync.dma_start(iw2_t[:], d_iw2[0])

        # ---- per-(layer, sample) pieces ----
        xrTs = list(x0Ts)
        xhTs = list(x0Ts)
        alphas = [None] * BSL
        a2bs = [None] * BSL
        xls = [None] * BSL
        xws = [None] * BSL
        xwrs = [None] * BSL
        an1s = [None] * BSL
        ctxT = None

        def alpha_chain(l, s):
            """Masked softmax over incident hyperedges -> alpha (for msg) and
            a2b = alpha * 0.25/D(v) (for the node-side aggregation).
            GpSimd builds the logits, Act does the exp, DVE the reductions."""
            an_src = an0s[s] if l == 0 else an1s[s]
            t1 = alph.tile([128, NB, HH, M], f32, tag="t1", bufs=2)
            nc.gpsimd.tensor_tensor(t1[:], _ins0(abs_[l][s][:], NB, 0),
                                    _ins0(an_src[:], M, -1), op=ALU.add)
            nc.vector.scalar_tensor_tensor(t1[:], t1[:], 0.2, t1[:],
                                           op0=ALU.mult, op1=ALU.max)
            nc.gpsimd.tensor_tensor(t1[:], t1[:], _ins0(hbs[s][:], HH, 1), op=ALU.add)
            nmax = alph.tile([128, NB, HH], f32, tag="nmax")
            nc.vector.tensor_reduce(nmax[:], t1[:], axis=AX.X, op=ALU.max, negate=True)
            nc.gpsimd.tensor_tensor(t1[:], t1[:], _ins0(nmax[:], M, -1), op=ALU.add)
            nc.scalar.activation(t1[:], t1[:], AF.Exp)
            ssum = alph.tile([128, NB, HH], f32, tag="ssum")
            nc.vector.tensor_reduce(ssum[:], t1[:], axis=AX.X, op=ALU.add)
            rs = alph.tile([128, NB, HH], f32, tag="rs")
            nc.vector.reciprocal(rs[:], ssum[:])
            rcol2 = alph.tile([128, NB, HH], f32, tag="rcol2")
            nc.vector.tensor_tensor(rcol2[:], rs[:], _ins0(ivdqs[s][:], HH, -1), op=ALU.mult)
            al = alph.tile([128, NB, HH, M], bf16, tag="alpha")
            nc.vector.tensor_tensor(al[:], t1[:], _ins0(rs[:], M, -1), op=ALU.mult)
            a2 = alph.tile([128, NB, HH, M], bf16, tag="a2b")
            nc.vector.tensor_tensor(a2[:], t1[:], _ins0(rcol2[:], M, -1), op=ALU.mult)
            alphas[s] = al
            a2bs[s] = a2

        def bulk(l, s):
            """x @ [w_lin | w_rel0 | w_rel1 | w_root]: all alpha-independent
            PE work for (l, s), drained chunk-by-chunk into bf16 SBUF."""
            xl = acts.tile([128, HH, NB, C], bf16, tag="xl")
            xw = acts.tile([128, NB, 2, C], bf16, tag="xw")
            xwr = acts.tile([128, NB, C], bf16, tag="xwr")
            for k in range(7):
                for nt in range(NB):
                    if k < HH:
                        stat = xhTs[s]
                        dst = xl[:, k, nt, :]
                    else:
                        stat = xrTs[s]
                        dst = xw[:, nt, k - 4, :] if k < 6 else xwr[:, nt, :]
                    ps = blk.tile([128, C], f32, tag="blk")
                    for ct in range(CT):
                        nc.tensor.matmul(ps[:], stat[:, ct, nt * 128 : (nt + 1) * 128],
                                         ws[l][:, k, ct, :], start=(ct == 0), stop=(ct == CT - 1))
                    copy(dst, ps[:])
            xls[s], xws[s], xwrs[s] = xl, xw, xwr

        def phase2(l, s):
            """alpha-dependent per-sample work: alpha transposes, msg,
            hypergraph out, RGCN aggregation, ctx column."""
            al, a2 = alphas[s], a2bs[s]
            xl, xw, xwr = xls[s], xws[s], xwrs[s]
            Afn = Afns[s]

            a3T = msgs.tile([M, HH, N], bf16, tag="a3T")
            for nb in range(NB):
                for h in range(HH):
                    tp = pmx.tile([M, 128], bf16, tag="mx", padded_shape=[M, 1024])
                    nc.tensor.transpose(tp[:], a2[:, nb, h, :], identb[:])
                    copy(a3T[:, h, nb * 128 : (nb + 1) * 128], tp[:])

            msg = msgs.tile([M, HH, C], bf16, tag="msg")
            for h in range(HH):
                mp = blk.tile([M, C], f32, tag="blk")
                for nb in range(NB):
                    nc.tensor.matmul(mp[:], al[:, nb, h, :], xl[:, h, nb, :],
                                     start=(nb == 0), stop=(nb == NB - 1))
                copy(msg[:, h, :], mp[:], scale=ivbs[s][:, 0:1])

            if l == 0:
                # transposed outputs -> next-layer state tiles [C, N]
                xhT1 = xst.tile([128, CT, N], bf16, tag="xst")
                xrT1 = xst.tile([128, CT, N], bf16, tag="xst")
                for g in range(CT // 2):
                    ph = pst.tile([128, 2, N], f32, tag="st")
                    for j in range(2):
                        ct = 2 * g + j
                        for h in range(HH):
                            nc.tensor.matmul(ph[:, j, :], msg[:, h, ct * 128 : (ct + 1) * 128],
                                             a3T[:, h, :], start=(h == 0), stop=(h == HH - 1))
                    for j in range(2):
                        ct = 2 * g + j
                        relu_bias(xhT1[:, ct, :], ph[:, j, :], bcolh[l][:, ct : ct + 1])
                for g in range(CT // 2):
                    pr = pst.tile([128, 2, N], f32, tag="st")
                    for j in range(2):
                        dt = 2 * g + j
                        first = True
                        for r in range(2):
                            for it in range(NB):
                                nc.tensor.matmul(pr[:, j, :],
                                                 xw[:, it, r, dt * 128 : (dt + 1) * 128],
                                                 Afn[:, r, it, :], start=first, stop=False)
                                first = False
                        for it in range(NB):
                            nc.tensor.matmul(pr[:, j, it * 128 : (it + 1) * 128],
                                             xwr[:, it, dt * 128 : (dt + 1) * 128],
                                             identb[:], start=False, stop=(it == NB - 1))
                    for j in range(2):
                        dt = 2 * g + j
                        relu_bias(xrT1[:, dt, :], pr[:, j, :], bcolr[l][:, dt : dt + 1])
                xhTs[s] = xhT1
                xrTs[s] = xrT1
                # ctx columns (pre-patch node-0 output)
                nc.vector.tensor_copy(ctxT[:, 0:CT, s : s + 1], xrT1[:, :, 0:1])
                nc.vector.tensor_copy(ctxT[:, CT : 2 * CT, s : s + 1], xhT1[:, :, 0:1])
            else:
                # row-major bf16 outputs, streamed to DRAM (rows 1..N-1)
                outh_t = outs.tile([128, NB, C], bf16, tag="outh")
                for nb in range(NB):
                    po = blk.tile([128, C], f32, tag="blk")
                    for h in range(HH):
                        nc.tensor.matmul(po[:], a3T[:, h, nb * 128 : (nb + 1) * 128],
                                         msg[:, h, :], start=(h == 0), stop=False)
                    nc.tensor.matmul(po[:], ones_row[:], browh[l][:], start=False, stop=True)
                    e = cpeng()
                    if e is nc.scalar:
                        e.activation(outh_t[:, nb, :], po[:], AF.Relu)
                    else:
                        e.tensor_scalar(outh_t[:, nb, :], po[:], 0.0, None, op0=ALU.max)
                outr_t = outs.tile([128, NB, C], bf16, tag="outr")
                for jb in range(NB):
                    po = blk.tile([128, C], f32, tag="blk")
                    first = True
                    for r in range(2):
                        for it in range(NB):
                            nc.tensor.matmul(po[:], Afn[:, r, it, jb * 128 : (jb + 1) * 128],
                                             xw[:, it, r, :], start=first, stop=False)
                            first = False
                    nc.tensor.matmul(po[:], identb[:], xwr[:, jb, :], start=False, stop=False)
                    nc.tensor.matmul(po[:], ones_row[:], browr[l][:], start=False, stop=True)
                    e = cpeng()
                    if e is nc.scalar:
                        e.activation(outr_t[:, jb, :], po[:], AF.Relu)
                    else:
                        e.tensor_scalar(outr_t[:, jb, :], po[:], 0.0, None, op0=ALU.max)
                # ctx rows -> columns
                cps = pmx.tile([128, 2, CT, 2], bf16, tag="mx", padded_shape=[128, 2, CT, 64])
                for ct in range(CT):
                    nc.tensor.transpose(cps[:, 0, ct, 0:1],
                                        outr_t[0:1, 0, ct * 128 : (ct + 1) * 128],
                                        identb[0:1, 0:1])
                    nc.tensor.transpose(cps[:, 1, ct, 0:1],
                                        outh_t[0:1, 0, ct * 128 : (ct + 1) * 128],
                                        identb[0:1, 0:1])
                nc.vector.tensor_copy(
                    ctxT[:, :, s : s + 1].rearrange("p (r ct) o -> p r (ct o)", r=2),
                    cps[:, :, :, 0])
                for tsrc, dram in ((outr_t, d_outr), (outh_t, d_outh)):
                    nc.sync.dma_start(dram[s, 1:128, :], tsrc[1:128, 0, :])
                    nc.sync.dma_start(dram[s, 128:N, :], tsrc[:, 1, :])

        def ie(l):
            """info-exchange MLP over the BSL ctx columns; layer 0 patches the
            state tiles' node-0 column, layer 1 DMAs the node-0 output rows."""
            y1 = iep.tile([BSL, C2], bf16, tag="y1")
            for ch in range(2):
                yp = blk.tile([BSL, C], f32, tag="blk")
                for kt in range(KT2):
                    nc.tensor.matmul(yp[:], ctxT[:, kt, :], iw1_t[:, kt, ch * C : (ch + 1) * C],
                                     start=(kt == 0), stop=False)
                nc.tensor.matmul(yp[:], ones4[:], ib1r[l][:, ch * C : (ch + 1) * C],
                                 start=False, stop=True)
                e = cpeng()
                if e is nc.scalar:
                    e.activation(y1[:, ch * C : (ch + 1) * C], yp[:], AF.Relu)
                else:
                    e.tensor_scalar(y1[:, ch * C : (ch + 1) * C], yp[:], 0.0, None, op0=ALU.max)
            c2T = iep.tile([128, KT2, BSL], bf16, tag="c2T")
            for kt in range(KT2):
                tp = pmx.tile([128, BSL], bf16, tag="mx", padded_shape=[128, 1024])
                nc.tensor.transpose(tp[:], y1[:, kt * 128 : (kt + 1) * 128], identb[0:BSL, 0:BSL])
                copy(c2T[:, kt, :], tp[:])
            y2 = iep.tile([BSL, C2], bf16, tag="y2")
            for ch in range(2):
                yp = blk.tile([BSL, C], f32, tag="blk")
                for kt in range(KT2):
                    nc.tensor.matmul(yp[:], c2T[:, kt, :], iw2_t[:, kt, ch * C : (ch + 1) * C],
                                     start=(kt == 0), stop=False)
                nc.tensor.matmul(yp[:], ones4[:], ib2r[l][:, ch * C : (ch + 1) * C],
                                 start=False, stop=True)
                copy(y2[:, ch * C : (ch + 1) * C], yp[:])
            if l == 0:
                for kt in range(KT2):
                    tp = pmx.tile([128, BSL], bf16, tag="mx", padded_shape=[128, 1024])
                    nc.tensor.transpose(tp[:], y2[:, kt * 128 : (kt + 1) * 128],
                                        identb[0:BSL, 0:BSL])
                    for s in range(BSL):
                        dst = xrTs[s] if kt < CT else xhTs[s]
                        e = cpeng()
                        if e is nc.scalar:
                            e.copy(dst[:, kt % CT, 0:1], tp[:, s : s + 1])
                        else:
                            e.tensor_copy(dst[:, kt % CT, 0:1], tp[:, s : s + 1])
            else:
                for s in range(BSL):
                    nc.sync.dma_start(d_outr[s, 0:1, :], y2[s : s + 1, 0:C])
                    nc.sync.dma_start(d_outh[s, 0:1, :], y2[s : s + 1, C:C2])

        # ================= layer 0 =================
        ctxT = iep.tile([128, 2 * CT, BSL], bf16, tag="ctxT")
        for s in range(BSL):
            alpha_chain(0, s)
        bulk(0, 0)
        bulk(0, 1)
        phase2(0, 0)
        bulk(0, 2)
        phase2(0, 1)
        bulk(0, 3)
        phase2(0, 2)
        phase2(0, 3)
        ie(0)

        # layer-1 IE weights reuse the same SBUF slots (gated on ie(0) readers)
        iw1_t = wie.tile([128, KT2, C2], bf16, tag="iw1")
        nc.gpsimd.dma_start(iw1_t[:], d_iw1[1])
        iw2_t = wie.tile([128, KT2, C2], bf16, tag="iw2")
        nc.sync.dma_start(iw2_t[:], d_iw2[1])

        # ================= layer 1 =================
        ctxT = iep.tile([128, 2 * CT, BSL], bf16, tag="ctxT2")
        for s in range(BSL):
            ap = blk.tile([HH, N], f32, tag="blk", padded_shape=[HH, 512])
            for ct in range(CT):
                nc.tensor.matmul(ap[:], ux1[:, ct, :], xhTs[s][:, ct, :],
                                 start=(ct == 0), stop=(ct == CT - 1))
            anrow = iep.tile([HH, N], bf16, tag="anrow", name=f"anrow_{s}")
            copy(anrow[:], ap[:])
            an1 = alph.tile([128, NB, HH], f32, tag="an1")
            for nb in range(NB):
                tp = pmx.tile([128, HH], bf16, tag="mx", padded_shape=[128, 1024])
                nc.tensor.transpose(tp[:], anrow[:, nb * 128 : (nb + 1) * 128],
                                    identb[0:HH, 0:HH])
                copy(an1[:, nb, :], tp[:])
            an1s[s] = an1
        for s in range(BSL):
            alpha_chain(1, s)
        bulk(1, 0)
        bulk(1, 1)
        phase2(1, 0)
        bulk(1, 2)
        phase2(1, 1)
        bulk(1, 3)
        phase2(1, 2)
        phase2(1, 3)
        ie(1)

    nc.compile()
    return nc


_NC = None


def _get_nc():
    global _NC
    if _NC is None:
        _NC = build_module()
    return _NC


def make_in_maps(encoded_spans, SVO_emb, pooled_output, sent2word_adj, aug_adj,
                 punct_graph, w_rel, w_root, b_rgcn, w_lin, att_x, att_e, b_hgcn,
                 ie_w1, ie_b1, ie_w2, ie_b2):
    f = np.float32
    bf = ml_dtypes.bfloat16
    x = np.asarray(encoded_spans, f)                               # [BS, N, C]
    aug = np.asarray(aug_adj, f)
    pun = np.asarray(punct_graph, f)
    A = np.stack([pun * (1.0 - aug), aug], axis=1)                 # [BS, 2, N, N]
    deg = A.sum(axis=2)                                            # in-degree of target j
    adjn = A / np.where(deg > 0, deg, 1.0)[:, :, None, :]
    Hinc = np.concatenate([np.ones((BS, N, 1), f),
                           np.asarray(sent2word_adj, f)], axis=2)  # [BS, N, M]
    hbm = np.where(Hinc > 0, 0.0, -50.0).astype(f)
    Dn = Hinc.sum(axis=2)
    ivdq = (0.25 / np.where(Dn > 0, Dn, 1.0)).astype(f)            # [BS, N]
    Be = Hinc.sum(axis=1)
    ivb = np.where(Be > 0, 1.0 / np.where(Be > 0, Be, 1.0), 0.0).astype(f)  # [BS, M]
    e_attr = np.concatenate([np.asarray(pooled_output, f)[:, None, :],
                             np.asarray(SVO_emb, f)], axis=1)      # [BS, M, C]
    wl = np.ascontiguousarray(np.asarray(w_lin, f))                # [L, C, HH*C]
    wl4 = wl.reshape(L, C, HH, C)
    ux = np.einsum("lchk,lhk->lch", wl4, np.asarray(att_x, f))     # [L, C, HH]
    ue = np.einsum("lchk,lhk->lch", wl4, np.asarray(att_e, f))
    an0 = np.einsum("bnc,ch->bnh", x, ux[0]).astype(f)             # [BS, N, HH]
    ae = np.einsum("bmc,lch->lbhm", e_attr, ue)                    # [L, BS, HH, M]
    wr = np.asarray(w_rel, f)
    wcat = np.concatenate([wr[:, 0], wr[:, 1], np.asarray(w_root, f)], axis=2)
    x0T = np.ascontiguousarray(x.transpose(0, 2, 1))               # [BS, C, N]

    # partition-major swizzles: index (ct*128+p) -> [p][ct]
    wcomb = np.concatenate([wl, wcat], axis=2)                     # [L, C, 3584]
    wcomb = wcomb.reshape(L, CT, 128, 7, 512).transpose(0, 2, 3, 1, 4)  # [L,128,7,CT,512]
    x0Tp = x0T.reshape(BS, CT, 128, N).transpose(0, 2, 1, 3)       # [BS,128,CT,N]
    adjnp = adjn.reshape(BS, 2, NB, 128, N).transpose(0, 3, 1, 2, 4)  # [BS,128,2,NB,N]
    hbp = hbm.reshape(BS, NB, 128, M).transpose(0, 2, 1, 3)        # [BS,128,NB,M]
    an0p = an0.reshape(BS, NB, 128, HH).transpose(0, 2, 1, 3)      # [BS,128,NB,HH]
    ivdqp = ivdq.reshape(BS, NB, 128).transpose(0, 2, 1)           # [BS,128,NB]
    iw1p = np.asarray(ie_w1, f).reshape(L, KT2, 128, C2).transpose(0, 2, 1, 3)
    iw2p = np.asarray(ie_w2, f).reshape(L, KT2, 128, C2).transpose(0, 2, 1, 3)
    ux1p = ux[1].reshape(CT, 128, HH).transpose(1, 0, 2)           # [128,CT,HH]
    bcolrp = np.asarray(b_rgcn, f).reshape(L, CT, 128).transpose(0, 2, 1)
    bcolhp = np.asarray(b_hgcn, f).reshape(L, CT, 128).transpose(0, 2, 1)

    shared = {
        "wcomb": np.ascontiguousarray(wcomb).astype(bf),
        "ux1": np.ascontiguousarray(ux1p).astype(bf),
        "iw1": np.ascontiguousarray(iw1p).astype(bf),
        "iw2": np.ascontiguousarray(iw2p).astype(bf),
        "bcolr": np.ascontiguousarray(bcolrp),
        "bcolh": np.ascontiguousarray(bcolhp),
        "browr": np.asarray(b_rgcn, f).astype(bf),
        "browh": np.asarray(b_hgcn, f).astype(bf),
        "ib1": np.asarray(ie_b1, f).astype(bf),
        "ib2": np.asarray(ie_b2, f).astype(bf),
        "eyeb": np.eye(128, dtype=f).astype(bf),
        "onesr": np.ones((1, 128), f).astype(bf),
        "ones4": np.ones((1, 4), f).astype(bf),
    }
    in_maps = []
    for c in range(NCORES):
        sl = slice(c * BSL, (c + 1) * BSL)
        m = dict(shared)
        m["x0T"] = np.ascontiguousarray(x0Tp[sl]).astype(bf)
        m["adjn"] = np.ascontiguousarray(adjnp[sl]).astype(bf)
        m["hbm"] = np.ascontiguousarray(hbp[sl]).astype(bf)
        m["ivdq"] = np.ascontiguousarray(ivdqp[sl])
        m["ivb"] = np.ascontiguousarray(ivb[sl])[:, :, None]
        m["an0"] = np.ascontiguousarray(an0p[sl])
        m["ae"] = np.ascontiguousarray(ae[:, sl]).astype(bf)
        in_maps.append(m)
    return in_maps


def run(in_maps, trace=False, **kw):
    nc = _get_nc()
    return run_bass_kernel_spmd(nc, in_maps, list(range(NCORES)), trace=trace, **kw)


def kernel(**inputs):
    in_maps = make_in_maps(**inputs)
    res = run(in_maps)
    x_r = np.concatenate([np.asarray(res.results[c]["outr"]) for c in range(NCORES)], axis=0)
    x_h = np.concatenate([np.asarray(res.results[c]["outh"]) for c in range(NCORES)], axis=0)
    return x_r.astype(np.float32), x_h.astype(np.float32)
